# revision 10
# baseline (speedup 1.0000x reference)
"""AtomConvLayer (CGCNN message passing) distributed Bass kernel for 8 TRN2 NeuronCores.

Strategy (data-parallel over atoms N):
  - Each core owns N/8 = 6250 atom rows (padded to 6272 = 49*128).
  - The neighbor-feature gather atom_in_fea[nbr_fea_idx] is done on-device with
    `dma_gather` against a replicated bf16 *pair* table: dma_gather requires
    int16 indices (< 32768), so rows are addressed in pairs (25000 pairs) and the
    wrong-parity half of each gathered pair is zeroed via interleaved weights.
  - Per 128-row tile: w = bwi*bwj, self/nbr/bond parts reduced on DVE,
    concat -> PE transpose -> matmul with W^T -> z kept in SBUF.
  - BatchNorm stats are partition-reduced per tile with a valid-mask matmul,
    accumulated in PSUM, then AllReduce'd across the 8 cores (sum, sumsq).
  - BN1 affine + sigmoid*softplus -> second stats AllReduce -> BN2 + softplus.
"""
import sys

sys.path.insert(0, "/opt/trn_rl_repo")

import numpy as np
import ml_dtypes

from concourse import bass, bacc, mybir, tile
from concourse.bass_utils import run_bass_kernel_spmd

# problem sizes (hardcoded per spec)
N = 50000
M = 24
A = 64  # atom_fea_len
B = 32  # nbr_fea_len
F = 2 * A + B  # 160
J = 2 * A  # 128
EPS = 1e-5

CORES = 8
P = 128
NS = N // CORES  # 6250
T = (NS + P - 1) // P  # 49 tiles
NPAD = T * P  # 6272
NPAIR = N // 2  # 25000
IW = (M * P) // 16  # 192  idx free width per tile

f32 = mybir.dt.float32
bf16 = mybir.dt.bfloat16
i16 = mybir.dt.int16
i32 = mybir.dt.int32


def build_graph(n=N, m=M, cores=CORES):
    """Build the SPMD Tile graph. Parameterized so a scaled-down version can be
    simulated; the real kernel uses the module constants."""
    ns = n // cores
    t_tiles = (ns + P - 1) // P
    npad = t_tiles * P
    npair = n // 2
    iw = (m * P) // 16
    m2 = 2 * m

    nc_ = bacc.Bacc("TRN2", target_bir_lowering=False, debug=False, num_devices=cores)
    tc = tile.TileContext(nc_)
    tc.__enter__()
    nc = tc.nc

    # ---- DRAM parameters (per-core shards supplied via in_maps) ----
    atab_d = nc.dram_tensor("atab", [n, A], f32, kind="ExternalInput")
    aself_d = nc.dram_tensor("aself", [npad, A], f32, kind="ExternalInput")
    nbr_d = nc.dram_tensor("nbr", [npad, m * B], f32, kind="ExternalInput")
    bwi_d = nc.dram_tensor("bwi", [npad, m], f32, kind="ExternalInput")
    bwj_d = nc.dram_tensor("bwj", [npad, m], f32, kind="ExternalInput")
    idx_d = nc.dram_tensor("idx", [npad, m], i32, kind="ExternalInput")
    validT_d = nc.dram_tensor("validT", [P, t_tiles], f32, kind="ExternalInput")
    wt1_d = nc.dram_tensor("wt1", [J, J], f32, kind="ExternalInput")
    wt2_d = nc.dram_tensor("wt2", [B, J], f32, kind="ExternalInput")
    g1b1_d = nc.dram_tensor("g1b1", [2, J], f32, kind="ExternalInput")
    g2b2_d = nc.dram_tensor("g2b2", [2, A], f32, kind="ExternalInput")
    ident_d = nc.dram_tensor("ident", [P, P], f32, kind="ExternalInput")
    out_d = nc.dram_tensor("out", [npad, A], f32, kind="ExternalOutput")

    rg = [list(range(cores))]

    from contextlib import ExitStack

    es_main = ExitStack()
    const = es_main.enter_context(tc.tile_pool(name="const", bufs=1))
    persist = es_main.enter_context(tc.tile_pool(name="persist", bufs=1))

    # constants
    ident = const.tile([P, P], f32)
    nc.sync.dma_start(out=ident[:], in_=ident_d[:])
    ones1 = const.tile([1, P], f32)
    nc.vector.memset(ones1[:], 1.0)
    wt1_sb = const.tile([J, J], f32)
    nc.sync.dma_start(out=wt1_sb[:], in_=wt1_d[:])
    wt2_sb = const.tile([B, J], f32)
    nc.sync.dma_start(out=wt2_sb[:], in_=wt2_d[:])
    gamma1_sb = const.tile([1, J], f32)
    nc.sync.dma_start(out=gamma1_sb[:], in_=g1b1_d[0:1, :])
    beta1_sb = const.tile([1, J], f32)
    nc.sync.dma_start(out=beta1_sb[:], in_=g1b1_d[1:2, :])
    gamma2_sb = const.tile([1, A], f32)
    nc.sync.dma_start(out=gamma2_sb[:], in_=g2b2_d[0:1, :])
    beta2_sb = const.tile([1, A], f32)
    nc.sync.dma_start(out=beta2_sb[:], in_=g2b2_d[1:2, :])
    validT_sb = const.tile([P, t_tiles], f32)
    nc.sync.dma_start(out=validT_sb[:], in_=validT_d[:])

    # persistent activations
    z_all = persist.tile([P, t_tiles * J], f32)
    core_all = persist.tile([P, t_tiles * A], f32)

    # ---------------- phase 1: message passing + linear + BN1 stats ----------
    es1 = ExitStack()
    ph1 = es1.enter_context(tc.tile_pool(name="ph1", bufs=2))
    psum1 = es1.enter_context(tc.tile_pool(name="psum1", bufs=2, space="PSUM"))
    psum_acc = es1.enter_context(tc.tile_pool(name="psum_acc", bufs=1, space="PSUM"))

    statz = psum_acc.tile([1, J], f32, name="statz")
    statz2 = psum_acc.tile([1, J], f32, name="statz2")

    for t in range(t_tiles):
        r0 = t * P
        # loads
        idx_sb = ph1.tile([P, m], i32, name="idx_sb")
        nc.sync.dma_start(out=idx_sb[:], in_=idx_d[r0 : r0 + P, :])
        bwi_sb = ph1.tile([P, m], f32, name="bwi_sb")
        nc.sync.dma_start(out=bwi_sb[:], in_=bwi_d[r0 : r0 + P, :])
        bwj_sb = ph1.tile([P, m], f32, name="bwj_sb")
        nc.sync.dma_start(out=bwj_sb[:], in_=bwj_d[r0 : r0 + P, :])
        aself_sb = ph1.tile([P, A], f32, name="aself_sb")
        nc.sync.dma_start(out=aself_sb[:], in_=aself_d[r0 : r0 + P, :])
        nbr_sb = ph1.tile([P, m * B], f32, name="nbr_sb")
        nc.sync.dma_start(out=nbr_sb[:], in_=nbr_d[r0 : r0 + P, :])

        # gather: G[p, c, :] = atab[idx[p, c], :]   (HW: one offset per partition
        # per indirect DMA, so one call per neighbor column)
        G = ph1.tile([P, m * A], f32, name="G")
        Gv = G[:].rearrange("p (c e) -> p c e", e=A)
        for c in range(m):
            nc.gpsimd.indirect_dma_start(
                out=Gv[:, c, :],
                out_offset=None,
                in_=atab_d[:],
                in_offset=bass.IndirectOffsetOnAxis(ap=idx_sb[:, c : c + 1], axis=0),
            )

        # w = bwi * bwj ; s = sum_m w
        w_sb = ph1.tile([P, m], f32, name="w_sb")
        nc.vector.tensor_tensor(out=w_sb[:], in0=bwi_sb[:], in1=bwj_sb[:], op=mybir.AluOpType.mult)
        s_sb = ph1.tile([P, 1], f32, name="s_sb")
        nc.vector.reduce_sum(out=s_sb[:], in_=w_sb[:], axis=mybir.AxisListType.X)

        tg = ph1.tile([P, F], f32, name="tg")

        # self part: tg[:, :A] = aself * s   (ACT per-partition scale)
        nc.scalar.mul(tg[:, 0:A], aself_sb[:], s_sb[:, 0:1])

        # neighbor part: prod[p, g, a] = G[p, g, a] * w[p, g] ; sum over g (24)
        prod = ph1.tile([P, m * A], f32, name="prod")
        nc.vector.tensor_tensor(
            out=prod[:],
            in0=G[:],
            in1=w_sb[:].unsqueeze(2).to_broadcast([P, m, A]),
            op=mybir.AluOpType.mult,
        )
        # reduce tree over g: 24 -> 12 -> 6 -> 3 -> 1
        src = prod[:].rearrange("p (g a) -> p g a", a=A)
        g_cnt = m
        lvl_i = 0
        while g_cnt > 3:
            half = g_cnt // 2
            nxt = ph1.tile([P, half * A], f32, name=f"nlvl{lvl_i}", tag=f"nlvl{lvl_i}")
            nc.vector.tensor_tensor(
                out=nxt[:], in0=src[:, 0:half, :], in1=src[:, half : 2 * half, :],
                op=mybir.AluOpType.add,
            )
            src = nxt[:].rearrange("p (g a) -> p g a", a=A)
            g_cnt = half
            lvl_i += 1
        assert g_cnt == 3
        nl = ph1.tile([P, A], f32, name="nl")
        nc.vector.tensor_tensor(out=nl[:], in0=src[:, 0, :], in1=src[:, 1, :], op=mybir.AluOpType.add)
        nc.vector.tensor_tensor(out=tg[:, A : 2 * A], in0=nl[:], in1=src[:, 2, :], op=mybir.AluOpType.add)

        # bond part: bprod[p, mm, b] = nbr[p, mm, b] * w[p, mm]; sum over mm (24)
        bprod = ph1.tile([P, m * B], f32, name="bprod")
        nc.vector.tensor_tensor(
            out=bprod[:],
            in0=nbr_sb[:],
            in1=w_sb[:].unsqueeze(2).to_broadcast([P, m, B]),
            op=mybir.AluOpType.mult,
        )
        bsrc = bprod[:].rearrange("p (g b) -> p g b", b=B)
        g_cnt = m
        lvl_i = 0
        while g_cnt > 3:
            half = g_cnt // 2
            nxt = ph1.tile([P, half * B], f32, name=f"blvl{lvl_i}", tag=f"blvl{lvl_i}")
            nc.vector.tensor_tensor(
                out=nxt[:], in0=bsrc[:, 0:half, :], in1=bsrc[:, half : 2 * half, :],
                op=mybir.AluOpType.add,
            )
            bsrc = nxt[:].rearrange("p (g b) -> p g b", b=B)
            g_cnt = half
            lvl_i += 1
        assert g_cnt == 3
        bl = ph1.tile([P, B], f32, name="bl")
        nc.vector.tensor_tensor(out=bl[:], in0=bsrc[:, 0, :], in1=bsrc[:, 1, :], op=mybir.AluOpType.add)
        nc.vector.tensor_tensor(out=tg[:, 2 * A : F], in0=bl[:], in1=bsrc[:, 2, :], op=mybir.AluOpType.add)

        # transpose tg -> tgT (two chunks), then z = tg @ W^T
        pT1 = psum1.tile([P, P], f32, name="pT1")
        nc.tensor.transpose(out=pT1[:], in_=tg[:, 0:J], identity=ident[:])
        pT2 = psum1.tile([B, P], f32, name="pT2")
        nc.tensor.transpose(out=pT2[:], in_=tg[:, J:F], identity=ident[:])
        tgT1 = ph1.tile([P, P], f32, name="tgT1")
        nc.vector.tensor_copy(out=tgT1[:], in_=pT1[:])
        tgT2 = ph1.tile([B, P], f32, name="tgT2")
        nc.vector.tensor_copy(out=tgT2[:], in_=pT2[:])

        zp = psum1.tile([P, J], f32, name="zp")
        nc.tensor.matmul(out=zp[:], lhsT=tgT1[:], rhs=wt1_sb[:], start=True, stop=False)
        nc.tensor.matmul(out=zp[:], lhsT=tgT2[:], rhs=wt2_sb[:], start=False, stop=True)

        z_sl = z_all[:, t * J : (t + 1) * J]
        nc.vector.tensor_copy(out=z_sl, in_=zp[:])
        z2_sb = ph1.tile([P, J], f32, name="z2_sb")
        nc.scalar.square(z2_sb[:], zp[:])

        # BN1 partial stats (masked partition sums, accumulated in PSUM)
        vcol = validT_sb[:, t : t + 1]
        nc.tensor.matmul(
            out=statz[:], lhsT=vcol, rhs=z_sl, start=(t == 0), stop=(t == t_tiles - 1),
            skip_group_check=True,
        )
        nc.tensor.matmul(
            out=statz2[:], lhsT=vcol, rhs=z2_sb[:], start=(t == 0), stop=(t == t_tiles - 1),
            skip_group_check=True,
        )

    # ---- AllReduce BN1 stats ----
    sz_sb = persist.tile([1, J], f32)
    nc.vector.tensor_copy(out=sz_sb[:], in_=statz[:])
    sz2_sb = persist.tile([1, J], f32)
    nc.vector.tensor_copy(out=sz2_sb[:], in_=statz2[:])

    dram = es_main.enter_context(tc.tile_pool(name="dram", bufs=1, space="DRAM"))
    ar1_in = dram.tile([2, J], f32)
    ar1_out = dram.tile([2, J], f32, addr_space="Shared")
    nc.sync.dma_start(out=ar1_in[0:1, :], in_=sz_sb[:])
    nc.sync.dma_start(out=ar1_in[1:2, :], in_=sz2_sb[:])
    nc.gpsimd.collective_compute(
        "AllReduce", mybir.AluOpType.add, replica_groups=rg,
        ins=[ar1_in[:].opt()], outs=[ar1_out[:].opt()],
    )
    sum1g = persist.tile([1, J], f32)
    nc.sync.dma_start(out=sum1g[:], in_=ar1_out[0:1, :])
    sq1g = persist.tile([1, J], f32)
    nc.sync.dma_start(out=sq1g[:], in_=ar1_out[1:2, :])

    # ---- BN1 affine coefficients + broadcast ----
    es1.close()

    coef = es_main.enter_context(tc.tile_pool(name="coef", bufs=1))
    psum_b = es_main.enter_context(tc.tile_pool(name="psum_b", bufs=1, space="PSUM"))

    inv_n = 1.0 / float(n)
    mean1 = coef.tile([1, J], f32)
    nc.scalar.mul(mean1[:], sum1g[:], inv_n)
    ex2 = coef.tile([1, J], f32)
    nc.scalar.mul(ex2[:], sq1g[:], inv_n)
    msq = coef.tile([1, J], f32)
    nc.vector.tensor_tensor(out=msq[:], in0=mean1[:], in1=mean1[:], op=mybir.AluOpType.mult)
    var1 = coef.tile([1, J], f32)
    nc.vector.tensor_tensor(out=var1[:], in0=ex2[:], in1=msq[:], op=mybir.AluOpType.subtract)
    nc.vector.tensor_scalar_add(var1[:], var1[:], EPS)
    lnv1 = coef.tile([1, J], f32)
    nc.scalar.activation(lnv1[:], var1[:], mybir.ActivationFunctionType.Ln)
    rstd1 = coef.tile([1, J], f32)
    nc.scalar.activation(rstd1[:], lnv1[:], mybir.ActivationFunctionType.Exp, scale=-0.5)
    # a1 = gamma1 * rstd ; c1 = beta1 - mean * a1  (packed [1, 2J])
    a1c1 = coef.tile([1, 2 * J], f32)
    nc.vector.tensor_tensor(out=a1c1[:, 0:J], in0=gamma1_sb[:], in1=rstd1[:], op=mybir.AluOpType.mult)
    ma1 = coef.tile([1, J], f32)
    nc.vector.tensor_tensor(out=ma1[:], in0=mean1[:], in1=a1c1[:, 0:J], op=mybir.AluOpType.mult)
    nc.vector.tensor_tensor(out=a1c1[:, J : 2 * J], in0=beta1_sb[:], in1=ma1[:], op=mybir.AluOpType.subtract)

    bc1p = psum_b.tile([P, 2 * J], f32)
    nc.tensor.matmul(out=bc1p[:], lhsT=ones1[:], rhs=a1c1[:], start=True, stop=True)
    A1C1 = persist.tile([P, 2 * J], f32)
    nc.vector.tensor_copy(out=A1C1[:], in_=bc1p[:])

    # ---------------- phase 2: BN1 apply + gating + BN2 stats ----------------
    es2 = ExitStack()
    ph2 = es2.enter_context(tc.tile_pool(name="ph2", bufs=2))
    psum2 = es2.enter_context(tc.tile_pool(name="psum2", bufs=1, space="PSUM"))
    statc = psum2.tile([1, A], f32, name="statc")
    statc2 = psum2.tile([1, A], f32, name="statc2")

    for t in range(t_tiles):
        z_sl = z_all[:, t * J : (t + 1) * J]
        zn = ph2.tile([P, J], f32, name="zn")
        nc.vector.tensor_tensor(out=zn[:], in0=z_sl, in1=A1C1[:, 0:J], op=mybir.AluOpType.mult)
        nc.vector.tensor_tensor(out=zn[:], in0=zn[:], in1=A1C1[:, J : 2 * J], op=mybir.AluOpType.add)
        en = ph2.tile([P, A], f32, name="en")
        nc.scalar.activation(en[:], zn[:, 0:A], mybir.ActivationFunctionType.Exp, scale=-1.0)
        nc.vector.tensor_scalar_add(en[:], en[:], 1.0)
        sig = ph2.tile([P, A], f32, name="sig")
        nc.vector.reciprocal(sig[:], en[:])
        ep = ph2.tile([P, A], f32, name="ep")
        nc.scalar.activation(ep[:], zn[:, A:J], mybir.ActivationFunctionType.Exp)
        sp = ph2.tile([P, A], f32, name="sp")
        nc.scalar.activation(sp[:], ep[:], mybir.ActivationFunctionType.Ln, bias=1.0)
        c_sl = core_all[:, t * A : (t + 1) * A]
        nc.vector.tensor_tensor(out=c_sl, in0=sig[:], in1=sp[:], op=mybir.AluOpType.mult)
        c2 = ph2.tile([P, A], f32, name="c2")
        nc.scalar.square(c2[:], c_sl)
        vcol = validT_sb[:, t : t + 1]
        nc.tensor.matmul(
            out=statc[:], lhsT=vcol, rhs=c_sl, start=(t == 0), stop=(t == t_tiles - 1),
            skip_group_check=True,
        )
        nc.tensor.matmul(
            out=statc2[:], lhsT=vcol, rhs=c2[:], start=(t == 0), stop=(t == t_tiles - 1),
            skip_group_check=True,
        )

    # ---- AllReduce BN2 stats ----
    sc_sb = persist.tile([1, A], f32)
    nc.vector.tensor_copy(out=sc_sb[:], in_=statc[:])
    sc2_sb = persist.tile([1, A], f32)
    nc.vector.tensor_copy(out=sc2_sb[:], in_=statc2[:])
    ar2_in = dram.tile([2, A], f32)
    ar2_out = dram.tile([2, A], f32, addr_space="Shared")
    nc.sync.dma_start(out=ar2_in[0:1, :], in_=sc_sb[:])
    nc.sync.dma_start(out=ar2_in[1:2, :], in_=sc2_sb[:])
    nc.gpsimd.collective_compute(
        "AllReduce", mybir.AluOpType.add, replica_groups=rg,
        ins=[ar2_in[:].opt()], outs=[ar2_out[:].opt()],
    )
    sum2g = persist.tile([1, A], f32)
    nc.sync.dma_start(out=sum2g[:], in_=ar2_out[0:1, :])
    sq2g = persist.tile([1, A], f32)
    nc.sync.dma_start(out=sq2g[:], in_=ar2_out[1:2, :])

    mean2 = coef.tile([1, A], f32)
    nc.scalar.mul(mean2[:], sum2g[:], inv_n)
    ex22 = coef.tile([1, A], f32)
    nc.scalar.mul(ex22[:], sq2g[:], inv_n)
    msq2 = coef.tile([1, A], f32)
    nc.vector.tensor_tensor(out=msq2[:], in0=mean2[:], in1=mean2[:], op=mybir.AluOpType.mult)
    var2 = coef.tile([1, A], f32)
    nc.vector.tensor_tensor(out=var2[:], in0=ex22[:], in1=msq2[:], op=mybir.AluOpType.subtract)
    nc.vector.tensor_scalar_add(var2[:], var2[:], EPS)
    lnv2 = coef.tile([1, A], f32)
    nc.scalar.activation(lnv2[:], var2[:], mybir.ActivationFunctionType.Ln)
    rstd2 = coef.tile([1, A], f32)
    nc.scalar.activation(rstd2[:], lnv2[:], mybir.ActivationFunctionType.Exp, scale=-0.5)
    a2c2 = coef.tile([1, 2 * A], f32)
    nc.vector.tensor_tensor(out=a2c2[:, 0:A], in0=gamma2_sb[:], in1=rstd2[:], op=mybir.AluOpType.mult)
    ma2 = coef.tile([1, A], f32)
    nc.vector.tensor_tensor(out=ma2[:], in0=mean2[:], in1=a2c2[:, 0:A], op=mybir.AluOpType.mult)
    nc.vector.tensor_tensor(out=a2c2[:, A : 2 * A], in0=beta2_sb[:], in1=ma2[:], op=mybir.AluOpType.subtract)

    bc2p = psum_b.tile([P, 2 * A], f32)
    nc.tensor.matmul(out=bc2p[:], lhsT=ones1[:], rhs=a2c2[:], start=True, stop=True)
    A2C2 = persist.tile([P, 2 * A], f32)
    nc.vector.tensor_copy(out=A2C2[:], in_=bc2p[:])

    # ---------------- phase 3: BN2 apply + softplus + store ----------------
    for t in range(t_tiles):
        c_sl = core_all[:, t * A : (t + 1) * A]
        cn = ph2.tile([P, A], f32, name="cn")
        nc.vector.tensor_tensor(out=cn[:], in0=c_sl, in1=A2C2[:, 0:A], op=mybir.AluOpType.mult)
        nc.vector.tensor_tensor(out=cn[:], in0=cn[:], in1=A2C2[:, A : 2 * A], op=mybir.AluOpType.add)
        ec = ph2.tile([P, A], f32, name="ec")
        nc.scalar.activation(ec[:], cn[:], mybir.ActivationFunctionType.Exp)
        ot = ph2.tile([P, A], f32, name="ot")
        nc.scalar.activation(ot[:], ec[:], mybir.ActivationFunctionType.Ln, bias=1.0)
        nc.sync.dma_start(out=out_d[t * P : (t + 1) * P, :], in_=ot[:])

    es2.close()
    es_main.close()
    tc.__exit__(None, None, None)
    nc_.compile()
    return nc_


def make_in_maps(inputs, n=N, m=M, cores=CORES):
    """Host-side sharding/layout prep (index reshuffling + dtype conversion only)."""
    ns = n // cores
    t_tiles = (ns + P - 1) // P
    npad = t_tiles * P

    atom = np.asarray(inputs["atom_in_fea"], np.float32)
    nbr = np.asarray(inputs["nbr_fea"], np.float32).reshape(n, m * B)
    idx = np.asarray(inputs["nbr_fea_idx"])
    bwi = np.asarray(inputs["bond_weights_i"], np.float32)
    bwj = np.asarray(inputs["bond_weights_j"], np.float32)
    W = np.asarray(inputs["W"], np.float32)
    g1 = np.asarray(inputs["gamma1"], np.float32)
    b1 = np.asarray(inputs["beta1"], np.float32)
    g2 = np.asarray(inputs["gamma2"], np.float32)
    b2 = np.asarray(inputs["beta2"], np.float32)

    Wt = np.ascontiguousarray(W.T)  # [F, J]
    wt1 = np.ascontiguousarray(Wt[0:J, :])
    wt2 = np.ascontiguousarray(Wt[J:F, :])
    g1b1 = np.stack([g1, b1]).astype(np.float32)
    g2b2 = np.stack([g2, b2]).astype(np.float32)

    valid = np.zeros((npad,), np.float32)
    valid[:ns] = 1.0
    validT = np.ascontiguousarray(valid.reshape(t_tiles, P).T)

    in_maps = []
    for c in range(cores):
        lo, hi = c * ns, (c + 1) * ns
        pad = npad - ns

        def padrows(x):
            return np.concatenate([x, np.zeros((pad,) + x.shape[1:], x.dtype)], 0) if pad else x

        idx_c = np.concatenate([idx[lo:hi], np.zeros((pad, m), idx.dtype)], 0) if pad else idx[lo:hi]

        in_maps.append(
            {
                "atab": atom,
                "aself": padrows(atom[lo:hi]),
                "nbr": padrows(nbr[lo:hi]),
                "bwi": padrows(bwi[lo:hi]),
                "bwj": padrows(bwj[lo:hi]),
                "idx": np.ascontiguousarray(idx_c.astype(np.int32)),
                "validT": validT,
                "wt1": wt1,
                "wt2": wt2,
                "g1b1": g1b1,
                "g2b2": g2b2,
                "ident": np.eye(P, dtype=np.float32),
            }
        )
    return in_maps


_GRAPH_CACHE = {}


def _get_graph():
    if "nc" not in _GRAPH_CACHE:
        _GRAPH_CACHE["nc"] = build_graph()
    return _GRAPH_CACHE["nc"]


def run(inputs, trace=False, **kw):
    nc = _get_graph()
    in_maps = make_in_maps(inputs)
    res = run_bass_kernel_spmd(nc, in_maps, core_ids=list(range(CORES)), trace=trace, **kw)
    ns = N // CORES
    out = np.concatenate([res.results[c]["out"][:ns] for c in range(CORES)], 0)
    return out.astype(np.float32), res


def kernel(**inputs) -> np.ndarray:
    out, _ = run(inputs, trace=False)
    return out


# revision 11
# speedup vs baseline: 1.2535x; 1.2535x over previous
"""AtomConvLayer (CGCNN message passing) distributed Bass kernel for 8 TRN2 NeuronCores.

Strategy (data-parallel over atoms N):
  - Each core owns N/8 = 6250 atom rows (padded to 6272 = 49*128).
  - The neighbor-feature gather atom_in_fea[nbr_fea_idx] is done on-device with
    `dma_gather` against a replicated bf16 *pair* table: dma_gather requires
    int16 indices (< 32768), so rows are addressed in pairs (25000 pairs) and the
    wrong-parity half of each gathered pair is zeroed via interleaved weights.
  - Per 128-row tile: w = bwi*bwj, self/nbr/bond parts reduced on DVE,
    concat -> PE transpose -> matmul with W^T -> z kept in SBUF.
  - BatchNorm stats are partition-reduced per tile with a valid-mask matmul,
    accumulated in PSUM, then AllReduce'd across the 8 cores (sum, sumsq).
  - BN1 affine + sigmoid*softplus -> second stats AllReduce -> BN2 + softplus.
"""
import sys

sys.path.insert(0, "/opt/trn_rl_repo")

import numpy as np
import ml_dtypes

from concourse import bass, bacc, mybir, tile
from concourse.bass_utils import run_bass_kernel_spmd

# problem sizes (hardcoded per spec)
N = 50000
M = 24
A = 64  # atom_fea_len
B = 32  # nbr_fea_len
F = 2 * A + B  # 160
J = 2 * A  # 128
EPS = 1e-5

CORES = 8
P = 128
NS = N // CORES  # 6250
T = (NS + P - 1) // P  # 49 tiles
NPAD = T * P  # 6272
NPAIR = N // 2  # 25000
IW = (M * P) // 16  # 192  idx free width per tile

f32 = mybir.dt.float32
bf16 = mybir.dt.bfloat16
i16 = mybir.dt.int16
i32 = mybir.dt.int32


def build_graph(n=N, m=M, cores=CORES):
    """Build the SPMD Tile graph. Parameterized so a scaled-down version can be
    simulated; the real kernel uses the module constants."""
    ns = n // cores
    t_tiles = (ns + P - 1) // P
    npad = t_tiles * P
    npair = n // 2
    iw = (m * P) // 16
    m2 = 2 * m

    nc_ = bacc.Bacc("TRN2", target_bir_lowering=False, debug=False, num_devices=cores)
    tc = tile.TileContext(nc_)
    tc.__enter__()
    nc = tc.nc

    # ---- DRAM parameters (per-core shards supplied via in_maps) ----
    atab_d = nc.dram_tensor("atab", [n, A], f32, kind="ExternalInput")
    aself_d = nc.dram_tensor("aself", [npad, A], f32, kind="ExternalInput")
    nbr_d = nc.dram_tensor("nbr", [npad, m * B], f32, kind="ExternalInput")
    bwi_d = nc.dram_tensor("bwi", [npad, m], f32, kind="ExternalInput")
    bwj_d = nc.dram_tensor("bwj", [npad, m], f32, kind="ExternalInput")
    idx_d = nc.dram_tensor("idx", [npad, m], i32, kind="ExternalInput")
    validT_d = nc.dram_tensor("validT", [P, t_tiles], f32, kind="ExternalInput")
    wt1_d = nc.dram_tensor("wt1", [J, J], f32, kind="ExternalInput")
    wt2_d = nc.dram_tensor("wt2", [B, J], f32, kind="ExternalInput")
    g1b1_d = nc.dram_tensor("g1b1", [2, J], f32, kind="ExternalInput")
    g2b2_d = nc.dram_tensor("g2b2", [2, A], f32, kind="ExternalInput")
    ident_d = nc.dram_tensor("ident", [P, P], f32, kind="ExternalInput")
    out_d = nc.dram_tensor("out", [npad, A], f32, kind="ExternalOutput")

    rg = [list(range(cores))]

    from contextlib import ExitStack

    es_main = ExitStack()
    const = es_main.enter_context(tc.tile_pool(name="const", bufs=1))
    persist = es_main.enter_context(tc.tile_pool(name="persist", bufs=1))

    # constants
    ident = const.tile([P, P], f32)
    nc.sync.dma_start(out=ident[:], in_=ident_d[:])
    ones1 = const.tile([1, P], f32)
    nc.vector.memset(ones1[:], 1.0)
    wt1_sb = const.tile([J, J], f32)
    nc.sync.dma_start(out=wt1_sb[:], in_=wt1_d[:])
    wt2_sb = const.tile([B, J], f32)
    nc.sync.dma_start(out=wt2_sb[:], in_=wt2_d[:])
    gamma1_sb = const.tile([1, J], f32)
    nc.sync.dma_start(out=gamma1_sb[:], in_=g1b1_d[0:1, :])
    beta1_sb = const.tile([1, J], f32)
    nc.sync.dma_start(out=beta1_sb[:], in_=g1b1_d[1:2, :])
    gamma2_sb = const.tile([1, A], f32)
    nc.sync.dma_start(out=gamma2_sb[:], in_=g2b2_d[0:1, :])
    beta2_sb = const.tile([1, A], f32)
    nc.sync.dma_start(out=beta2_sb[:], in_=g2b2_d[1:2, :])
    validT_sb = const.tile([P, t_tiles], f32)
    nc.sync.dma_start(out=validT_sb[:], in_=validT_d[:])

    # persistent activations
    z_all = persist.tile([P, t_tiles * J], f32)
    core_all = persist.tile([P, t_tiles * A], f32)
    sig_all = persist.tile([P, t_tiles * A], f32)
    ep_all = persist.tile([P, t_tiles * A], f32)
    ec_all = persist.tile([P, t_tiles * A], f32)

    # all gather indices resident up-front so the Pool queue never waits on Sync
    idx_all = persist.tile([P, t_tiles * m], i32)
    nc.sync.dma_start(
        out=idx_all[:].rearrange("p (t c) -> p t c", c=m),
        in_=idx_d[:].rearrange("(t p) c -> p t c", p=P),
    )

    # ---------------- phase 1: message passing + linear + BN1 stats ----------
    es1 = ExitStack()
    ph1 = es1.enter_context(tc.tile_pool(name="ph1", bufs=2))
    phg = es1.enter_context(tc.tile_pool(name="phg", bufs=3))
    psum1 = es1.enter_context(tc.tile_pool(name="psum1", bufs=2, space="PSUM"))
    psum_acc = es1.enter_context(tc.tile_pool(name="psum_acc", bufs=1, space="PSUM"))

    statz = psum_acc.tile([1, J], f32, name="statz")
    statz2 = psum_acc.tile([1, J], f32, name="statz2")

    for t in range(t_tiles):
        r0 = t * P
        # loads
        bwi_sb = ph1.tile([P, m], f32, name="bwi_sb")
        nc.sync.dma_start(out=bwi_sb[:], in_=bwi_d[r0 : r0 + P, :])
        bwj_sb = ph1.tile([P, m], f32, name="bwj_sb")
        nc.sync.dma_start(out=bwj_sb[:], in_=bwj_d[r0 : r0 + P, :])
        aself_sb = ph1.tile([P, A], f32, name="aself_sb")
        nc.sync.dma_start(out=aself_sb[:], in_=aself_d[r0 : r0 + P, :])
        nbr_sb = ph1.tile([P, m * B], f32, name="nbr_sb")
        nc.sync.dma_start(out=nbr_sb[:], in_=nbr_d[r0 : r0 + P, :])

        # gather: G[p, c, :] = atab[idx[p, c], :]   (HW: one offset per partition
        # per indirect DMA, so one call per neighbor column)
        G = phg.tile([P, m * A], f32, name="G")
        Gv = G[:].rearrange("p (c e) -> p c e", e=A)
        for c in range(m):
            nc.gpsimd.indirect_dma_start(
                out=Gv[:, c, :],
                out_offset=None,
                in_=atab_d[:],
                in_offset=bass.IndirectOffsetOnAxis(
                    ap=idx_all[:, t * m + c : t * m + c + 1], axis=0
                ),
            )

        # w = bwi * bwj ; s = sum_m w
        w_sb = ph1.tile([P, m], f32, name="w_sb")
        nc.vector.tensor_tensor(out=w_sb[:], in0=bwi_sb[:], in1=bwj_sb[:], op=mybir.AluOpType.mult)
        s_sb = ph1.tile([P, 1], f32, name="s_sb")
        nc.vector.reduce_sum(out=s_sb[:], in_=w_sb[:], axis=mybir.AxisListType.X)

        tg = ph1.tile([P, F], f32, name="tg")

        # self part: tg[:, :A] = aself * s   (ACT per-partition scale)
        nc.scalar.mul(tg[:, 0:A], aself_sb[:], s_sb[:, 0:1])

        # neighbor part: prod[p, g, a] = G[p, g, a] * w[p, g] ; sum over g (24)
        prod = ph1.tile([P, m * A], f32, name="prod")
        nc.vector.tensor_tensor(
            out=prod[:],
            in0=G[:],
            in1=w_sb[:].unsqueeze(2).to_broadcast([P, m, A]),
            op=mybir.AluOpType.mult,
        )
        # reduce tree over g: 24 -> 12 -> 6 -> 3 -> 1
        src = prod[:].rearrange("p (g a) -> p g a", a=A)
        g_cnt = m
        lvl_i = 0
        while g_cnt > 3:
            half = g_cnt // 2
            nxt = ph1.tile([P, half * A], f32, name=f"nlvl{lvl_i}", tag=f"nlvl{lvl_i}")
            nc.vector.tensor_tensor(
                out=nxt[:], in0=src[:, 0:half, :], in1=src[:, half : 2 * half, :],
                op=mybir.AluOpType.add,
            )
            src = nxt[:].rearrange("p (g a) -> p g a", a=A)
            g_cnt = half
            lvl_i += 1
        assert g_cnt == 3
        nl = ph1.tile([P, A], f32, name="nl")
        nc.vector.tensor_tensor(out=nl[:], in0=src[:, 0, :], in1=src[:, 1, :], op=mybir.AluOpType.add)
        nc.vector.tensor_tensor(out=tg[:, A : 2 * A], in0=nl[:], in1=src[:, 2, :], op=mybir.AluOpType.add)

        # bond part: bprod[p, mm, b] = nbr[p, mm, b] * w[p, mm]; sum over mm (24)
        bprod = ph1.tile([P, m * B], f32, name="bprod")
        nc.vector.tensor_tensor(
            out=bprod[:],
            in0=nbr_sb[:],
            in1=w_sb[:].unsqueeze(2).to_broadcast([P, m, B]),
            op=mybir.AluOpType.mult,
        )
        bsrc = bprod[:].rearrange("p (g b) -> p g b", b=B)
        g_cnt = m
        lvl_i = 0
        while g_cnt > 3:
            half = g_cnt // 2
            nxt = ph1.tile([P, half * B], f32, name=f"blvl{lvl_i}", tag=f"blvl{lvl_i}")
            nc.vector.tensor_tensor(
                out=nxt[:], in0=bsrc[:, 0:half, :], in1=bsrc[:, half : 2 * half, :],
                op=mybir.AluOpType.add,
            )
            bsrc = nxt[:].rearrange("p (g b) -> p g b", b=B)
            g_cnt = half
            lvl_i += 1
        assert g_cnt == 3
        bl = ph1.tile([P, B], f32, name="bl")
        nc.vector.tensor_tensor(out=bl[:], in0=bsrc[:, 0, :], in1=bsrc[:, 1, :], op=mybir.AluOpType.add)
        nc.vector.tensor_tensor(out=tg[:, 2 * A : F], in0=bl[:], in1=bsrc[:, 2, :], op=mybir.AluOpType.add)

        # transpose tg -> tgT (two chunks), then z = tg @ W^T
        pT1 = psum1.tile([P, P], f32, name="pT1")
        nc.tensor.transpose(out=pT1[:], in_=tg[:, 0:J], identity=ident[:])
        pT2 = psum1.tile([B, P], f32, name="pT2")
        nc.tensor.transpose(out=pT2[:], in_=tg[:, J:F], identity=ident[:])
        tgT1 = ph1.tile([P, P], f32, name="tgT1")
        nc.vector.tensor_copy(out=tgT1[:], in_=pT1[:])
        tgT2 = ph1.tile([B, P], f32, name="tgT2")
        nc.vector.tensor_copy(out=tgT2[:], in_=pT2[:])

        zp = psum1.tile([P, J], f32, name="zp")
        nc.tensor.matmul(out=zp[:], lhsT=tgT1[:], rhs=wt1_sb[:], start=True, stop=False)
        nc.tensor.matmul(out=zp[:], lhsT=tgT2[:], rhs=wt2_sb[:], start=False, stop=True)

        z_sl = z_all[:, t * J : (t + 1) * J]
        nc.vector.tensor_copy(out=z_sl, in_=zp[:])
        z2_sb = ph1.tile([P, J], f32, name="z2_sb")
        nc.scalar.square(z2_sb[:], zp[:])

        # BN1 partial stats (masked partition sums, accumulated in PSUM)
        vcol = validT_sb[:, t : t + 1]
        nc.tensor.matmul(
            out=statz[:], lhsT=vcol, rhs=z_sl, start=(t == 0), stop=(t == t_tiles - 1),
            skip_group_check=True,
        )
        nc.tensor.matmul(
            out=statz2[:], lhsT=vcol, rhs=z2_sb[:], start=(t == 0), stop=(t == t_tiles - 1),
            skip_group_check=True,
        )

    # ---- AllReduce BN1 stats ----
    sz_sb = persist.tile([1, J], f32)
    nc.vector.tensor_copy(out=sz_sb[:], in_=statz[:])
    sz2_sb = persist.tile([1, J], f32)
    nc.vector.tensor_copy(out=sz2_sb[:], in_=statz2[:])

    dram = es_main.enter_context(tc.tile_pool(name="dram", bufs=1, space="DRAM"))
    ar1_in = dram.tile([2, J], f32)
    ar1_out = dram.tile([2, J], f32, addr_space="Shared")
    nc.sync.dma_start(out=ar1_in[0:1, :], in_=sz_sb[:])
    nc.sync.dma_start(out=ar1_in[1:2, :], in_=sz2_sb[:])
    nc.gpsimd.collective_compute(
        "AllReduce", mybir.AluOpType.add, replica_groups=rg,
        ins=[ar1_in[:].opt()], outs=[ar1_out[:].opt()],
    )
    sum1g = persist.tile([1, J], f32)
    nc.sync.dma_start(out=sum1g[:], in_=ar1_out[0:1, :])
    sq1g = persist.tile([1, J], f32)
    nc.sync.dma_start(out=sq1g[:], in_=ar1_out[1:2, :])

    # ---- BN1 affine coefficients + broadcast ----
    es1.close()

    coef = es_main.enter_context(tc.tile_pool(name="coef", bufs=1))
    psum_b = es_main.enter_context(tc.tile_pool(name="psum_b", bufs=1, space="PSUM"))

    inv_n = 1.0 / float(n)
    mean1 = coef.tile([1, J], f32)
    nc.scalar.mul(mean1[:], sum1g[:], inv_n)
    ex2 = coef.tile([1, J], f32)
    nc.scalar.mul(ex2[:], sq1g[:], inv_n)
    msq = coef.tile([1, J], f32)
    nc.vector.tensor_tensor(out=msq[:], in0=mean1[:], in1=mean1[:], op=mybir.AluOpType.mult)
    var1 = coef.tile([1, J], f32)
    nc.vector.tensor_tensor(out=var1[:], in0=ex2[:], in1=msq[:], op=mybir.AluOpType.subtract)
    nc.vector.tensor_scalar_add(var1[:], var1[:], EPS)
    lnv1 = coef.tile([1, J], f32)
    nc.scalar.activation(lnv1[:], var1[:], mybir.ActivationFunctionType.Ln)
    rstd1 = coef.tile([1, J], f32)
    nc.scalar.activation(rstd1[:], lnv1[:], mybir.ActivationFunctionType.Exp, scale=-0.5)
    # a1 = gamma1 * rstd ; c1 = beta1 - mean * a1  (packed [1, 2J])
    a1c1 = coef.tile([1, 2 * J], f32)
    nc.vector.tensor_tensor(out=a1c1[:, 0:J], in0=gamma1_sb[:], in1=rstd1[:], op=mybir.AluOpType.mult)
    ma1 = coef.tile([1, J], f32)
    nc.vector.tensor_tensor(out=ma1[:], in0=mean1[:], in1=a1c1[:, 0:J], op=mybir.AluOpType.mult)
    nc.vector.tensor_tensor(out=a1c1[:, J : 2 * J], in0=beta1_sb[:], in1=ma1[:], op=mybir.AluOpType.subtract)

    bc1p = psum_b.tile([P, 2 * J], f32)
    nc.tensor.matmul(out=bc1p[:], lhsT=ones1[:], rhs=a1c1[:], start=True, stop=True)
    A1C1 = persist.tile([P, 2 * J], f32)
    nc.vector.tensor_copy(out=A1C1[:], in_=bc1p[:])

    # ---------------- phase 2: BN1 apply + gating + BN2 stats ----------------
    es2 = ExitStack()
    ph2 = es2.enter_context(tc.tile_pool(name="ph2", bufs=2))
    psum2 = es2.enter_context(tc.tile_pool(name="psum2", bufs=1, space="PSUM"))
    statc = psum2.tile([1, A], f32, name="statc")
    statc2 = psum2.tile([1, A], f32, name="statc2")

    # loop A: everything needing the Exp table (+ DVE sigmoid finish)
    for t in range(t_tiles):
        z_sl = z_all[:, t * J : (t + 1) * J]
        zn = ph2.tile([P, J], f32, name="zn")
        nc.vector.tensor_tensor(out=zn[:], in0=z_sl, in1=A1C1[:, 0:J], op=mybir.AluOpType.mult)
        nc.vector.tensor_tensor(out=zn[:], in0=zn[:], in1=A1C1[:, J : 2 * J], op=mybir.AluOpType.add)
        en = ph2.tile([P, A], f32, name="en")
        nc.scalar.activation(en[:], zn[:, 0:A], mybir.ActivationFunctionType.Exp, scale=-1.0)
        nc.vector.tensor_scalar_add(en[:], en[:], 1.0)
        nc.vector.reciprocal(sig_all[:, t * A : (t + 1) * A], en[:])
        nc.scalar.activation(
            ep_all[:, t * A : (t + 1) * A], zn[:, A:J], mybir.ActivationFunctionType.Exp
        )
    # loop B: Ln table (softplus finish), gating product, BN2 stats
    for t in range(t_tiles):
        sp = ph2.tile([P, A], f32, name="sp")
        nc.scalar.activation(
            sp[:], ep_all[:, t * A : (t + 1) * A], mybir.ActivationFunctionType.Ln, bias=1.0
        )
        c_sl = core_all[:, t * A : (t + 1) * A]
        nc.vector.tensor_tensor(
            out=c_sl, in0=sig_all[:, t * A : (t + 1) * A], in1=sp[:], op=mybir.AluOpType.mult
        )
        c2 = ph2.tile([P, A], f32, name="c2")
        nc.scalar.square(c2[:], c_sl)
        vcol = validT_sb[:, t : t + 1]
        nc.tensor.matmul(
            out=statc[:], lhsT=vcol, rhs=c_sl, start=(t == 0), stop=(t == t_tiles - 1),
            skip_group_check=True,
        )
        nc.tensor.matmul(
            out=statc2[:], lhsT=vcol, rhs=c2[:], start=(t == 0), stop=(t == t_tiles - 1),
            skip_group_check=True,
        )

    # ---- AllReduce BN2 stats ----
    sc_sb = persist.tile([1, A], f32)
    nc.vector.tensor_copy(out=sc_sb[:], in_=statc[:])
    sc2_sb = persist.tile([1, A], f32)
    nc.vector.tensor_copy(out=sc2_sb[:], in_=statc2[:])
    ar2_in = dram.tile([2, A], f32)
    ar2_out = dram.tile([2, A], f32, addr_space="Shared")
    nc.sync.dma_start(out=ar2_in[0:1, :], in_=sc_sb[:])
    nc.sync.dma_start(out=ar2_in[1:2, :], in_=sc2_sb[:])
    nc.gpsimd.collective_compute(
        "AllReduce", mybir.AluOpType.add, replica_groups=rg,
        ins=[ar2_in[:].opt()], outs=[ar2_out[:].opt()],
    )
    sum2g = persist.tile([1, A], f32)
    nc.sync.dma_start(out=sum2g[:], in_=ar2_out[0:1, :])
    sq2g = persist.tile([1, A], f32)
    nc.sync.dma_start(out=sq2g[:], in_=ar2_out[1:2, :])

    mean2 = coef.tile([1, A], f32)
    nc.scalar.mul(mean2[:], sum2g[:], inv_n)
    ex22 = coef.tile([1, A], f32)
    nc.scalar.mul(ex22[:], sq2g[:], inv_n)
    msq2 = coef.tile([1, A], f32)
    nc.vector.tensor_tensor(out=msq2[:], in0=mean2[:], in1=mean2[:], op=mybir.AluOpType.mult)
    var2 = coef.tile([1, A], f32)
    nc.vector.tensor_tensor(out=var2[:], in0=ex22[:], in1=msq2[:], op=mybir.AluOpType.subtract)
    nc.vector.tensor_scalar_add(var2[:], var2[:], EPS)
    lnv2 = coef.tile([1, A], f32)
    nc.scalar.activation(lnv2[:], var2[:], mybir.ActivationFunctionType.Ln)
    rstd2 = coef.tile([1, A], f32)
    nc.scalar.activation(rstd2[:], lnv2[:], mybir.ActivationFunctionType.Exp, scale=-0.5)
    a2c2 = coef.tile([1, 2 * A], f32)
    nc.vector.tensor_tensor(out=a2c2[:, 0:A], in0=gamma2_sb[:], in1=rstd2[:], op=mybir.AluOpType.mult)
    ma2 = coef.tile([1, A], f32)
    nc.vector.tensor_tensor(out=ma2[:], in0=mean2[:], in1=a2c2[:, 0:A], op=mybir.AluOpType.mult)
    nc.vector.tensor_tensor(out=a2c2[:, A : 2 * A], in0=beta2_sb[:], in1=ma2[:], op=mybir.AluOpType.subtract)

    bc2p = psum_b.tile([P, 2 * A], f32)
    nc.tensor.matmul(out=bc2p[:], lhsT=ones1[:], rhs=a2c2[:], start=True, stop=True)
    A2C2 = persist.tile([P, 2 * A], f32)
    nc.vector.tensor_copy(out=A2C2[:], in_=bc2p[:])

    # ---------------- phase 3: BN2 apply + softplus + store ----------------
    for t in range(t_tiles):
        c_sl = core_all[:, t * A : (t + 1) * A]
        cn = ph2.tile([P, A], f32, name="cn")
        nc.vector.tensor_tensor(out=cn[:], in0=c_sl, in1=A2C2[:, 0:A], op=mybir.AluOpType.mult)
        nc.vector.tensor_tensor(out=cn[:], in0=cn[:], in1=A2C2[:, A : 2 * A], op=mybir.AluOpType.add)
        nc.scalar.activation(
            ec_all[:, t * A : (t + 1) * A], cn[:], mybir.ActivationFunctionType.Exp
        )
    for t in range(t_tiles):
        ot = ph2.tile([P, A], f32, name="ot")
        nc.scalar.activation(
            ot[:], ec_all[:, t * A : (t + 1) * A], mybir.ActivationFunctionType.Ln, bias=1.0
        )
        nc.sync.dma_start(out=out_d[t * P : (t + 1) * P, :], in_=ot[:])

    es2.close()
    es_main.close()
    tc.__exit__(None, None, None)
    nc_.compile()
    return nc_


def make_in_maps(inputs, n=N, m=M, cores=CORES):
    """Host-side sharding/layout prep (index reshuffling + dtype conversion only)."""
    ns = n // cores
    t_tiles = (ns + P - 1) // P
    npad = t_tiles * P

    atom = np.asarray(inputs["atom_in_fea"], np.float32)
    nbr = np.asarray(inputs["nbr_fea"], np.float32).reshape(n, m * B)
    idx = np.asarray(inputs["nbr_fea_idx"])
    bwi = np.asarray(inputs["bond_weights_i"], np.float32)
    bwj = np.asarray(inputs["bond_weights_j"], np.float32)
    W = np.asarray(inputs["W"], np.float32)
    g1 = np.asarray(inputs["gamma1"], np.float32)
    b1 = np.asarray(inputs["beta1"], np.float32)
    g2 = np.asarray(inputs["gamma2"], np.float32)
    b2 = np.asarray(inputs["beta2"], np.float32)

    Wt = np.ascontiguousarray(W.T)  # [F, J]
    wt1 = np.ascontiguousarray(Wt[0:J, :])
    wt2 = np.ascontiguousarray(Wt[J:F, :])
    g1b1 = np.stack([g1, b1]).astype(np.float32)
    g2b2 = np.stack([g2, b2]).astype(np.float32)

    valid = np.zeros((npad,), np.float32)
    valid[:ns] = 1.0
    validT = np.ascontiguousarray(valid.reshape(t_tiles, P).T)

    in_maps = []
    for c in range(cores):
        lo, hi = c * ns, (c + 1) * ns
        pad = npad - ns

        def padrows(x):
            return np.concatenate([x, np.zeros((pad,) + x.shape[1:], x.dtype)], 0) if pad else x

        idx_c = np.concatenate([idx[lo:hi], np.zeros((pad, m), idx.dtype)], 0) if pad else idx[lo:hi]

        in_maps.append(
            {
                "atab": atom,
                "aself": padrows(atom[lo:hi]),
                "nbr": padrows(nbr[lo:hi]),
                "bwi": padrows(bwi[lo:hi]),
                "bwj": padrows(bwj[lo:hi]),
                "idx": np.ascontiguousarray(idx_c.astype(np.int32)),
                "validT": validT,
                "wt1": wt1,
                "wt2": wt2,
                "g1b1": g1b1,
                "g2b2": g2b2,
                "ident": np.eye(P, dtype=np.float32),
            }
        )
    return in_maps


_GRAPH_CACHE = {}


def _get_graph():
    if "nc" not in _GRAPH_CACHE:
        _GRAPH_CACHE["nc"] = build_graph()
    return _GRAPH_CACHE["nc"]


def run(inputs, trace=False, **kw):
    nc = _get_graph()
    in_maps = make_in_maps(inputs)
    res = run_bass_kernel_spmd(nc, in_maps, core_ids=list(range(CORES)), trace=trace, **kw)
    ns = N // CORES
    out = np.concatenate([res.results[c]["out"][:ns] for c in range(CORES)], 0)
    return out.astype(np.float32), res


def kernel(**inputs) -> np.ndarray:
    out, _ = run(inputs, trace=False)
    return out


# revision 12
# speedup vs baseline: 1.3419x; 1.0706x over previous
"""AtomConvLayer (CGCNN message passing) distributed Bass kernel for 8 TRN2 NeuronCores.

Strategy (data-parallel over atoms N):
  - Each core owns N/8 = 6250 atom rows (padded to 6272 = 49*128).
  - The neighbor-feature gather atom_in_fea[nbr_fea_idx] is done on-device with
    `dma_gather` against a replicated bf16 *pair* table: dma_gather requires
    int16 indices (< 32768), so rows are addressed in pairs (25000 pairs) and the
    wrong-parity half of each gathered pair is zeroed via interleaved weights.
  - Per 128-row tile: w = bwi*bwj, self/nbr/bond parts reduced on DVE,
    concat -> PE transpose -> matmul with W^T -> z kept in SBUF.
  - BatchNorm stats are partition-reduced per tile with a valid-mask matmul,
    accumulated in PSUM, then AllReduce'd across the 8 cores (sum, sumsq).
  - BN1 affine + sigmoid*softplus -> second stats AllReduce -> BN2 + softplus.
"""
import sys

sys.path.insert(0, "/opt/trn_rl_repo")

import numpy as np
import ml_dtypes

from concourse import bass, bacc, mybir, tile
from concourse.bass_utils import run_bass_kernel_spmd

# problem sizes (hardcoded per spec)
N = 50000
M = 24
A = 64  # atom_fea_len
B = 32  # nbr_fea_len
F = 2 * A + B  # 160
J = 2 * A  # 128
EPS = 1e-5

CORES = 8
P = 128
NS = N // CORES  # 6250
T = (NS + P - 1) // P  # 49 tiles
NPAD = T * P  # 6272
NPAIR = N // 2  # 25000
IW = (M * P) // 16  # 192  idx free width per tile

f32 = mybir.dt.float32
bf16 = mybir.dt.bfloat16
i16 = mybir.dt.int16
i32 = mybir.dt.int32


def build_graph(n=N, m=M, cores=CORES):
    """Build the SPMD Tile graph. Parameterized so a scaled-down version can be
    simulated; the real kernel uses the module constants."""
    ns = n // cores
    t_tiles = (ns + P - 1) // P
    npad = t_tiles * P
    npair = n // 2
    iw = (m * P) // 16
    m2 = 2 * m

    nc_ = bacc.Bacc("TRN2", target_bir_lowering=False, debug=False, num_devices=cores)
    tc = tile.TileContext(nc_)
    tc.__enter__()
    nc = tc.nc

    # ---- DRAM parameters (per-core shards supplied via in_maps) ----
    atab_d = nc.dram_tensor("atab", [n, A], f32, kind="ExternalInput")
    aself_d = nc.dram_tensor("aself", [npad, A], f32, kind="ExternalInput")
    nbr_d = nc.dram_tensor("nbr", [npad, m * B], f32, kind="ExternalInput")
    bwi_d = nc.dram_tensor("bwi", [npad, m], f32, kind="ExternalInput")
    bwj_d = nc.dram_tensor("bwj", [npad, m], f32, kind="ExternalInput")
    idx_d = nc.dram_tensor("idx", [npad, m], i32, kind="ExternalInput")
    validT_d = nc.dram_tensor("validT", [P, t_tiles], f32, kind="ExternalInput")
    wt1_d = nc.dram_tensor("wt1", [J, J], f32, kind="ExternalInput")
    wt2_d = nc.dram_tensor("wt2", [B, J], f32, kind="ExternalInput")
    g1b1_d = nc.dram_tensor("g1b1", [2, J], f32, kind="ExternalInput")
    g2b2_d = nc.dram_tensor("g2b2", [2, A], f32, kind="ExternalInput")
    ident_d = nc.dram_tensor("ident", [P, P], f32, kind="ExternalInput")
    out_d = nc.dram_tensor("out", [npad, A], f32, kind="ExternalOutput")

    rg = [list(range(cores))]

    from contextlib import ExitStack

    es_main = ExitStack()
    const = es_main.enter_context(tc.tile_pool(name="const", bufs=1))
    persist = es_main.enter_context(tc.tile_pool(name="persist", bufs=1))

    # constants
    ident = const.tile([P, P], f32)
    nc.sync.dma_start(out=ident[:], in_=ident_d[:])
    ones1 = const.tile([1, P], f32)
    nc.vector.memset(ones1[:], 1.0)
    wt1_sb = const.tile([J, J], f32)
    nc.sync.dma_start(out=wt1_sb[:], in_=wt1_d[:])
    wt2_sb = const.tile([B, J], f32)
    nc.sync.dma_start(out=wt2_sb[:], in_=wt2_d[:])
    gamma1_sb = const.tile([1, J], f32)
    nc.sync.dma_start(out=gamma1_sb[:], in_=g1b1_d[0:1, :])
    beta1_sb = const.tile([1, J], f32)
    nc.sync.dma_start(out=beta1_sb[:], in_=g1b1_d[1:2, :])
    gamma2_sb = const.tile([1, A], f32)
    nc.sync.dma_start(out=gamma2_sb[:], in_=g2b2_d[0:1, :])
    beta2_sb = const.tile([1, A], f32)
    nc.sync.dma_start(out=beta2_sb[:], in_=g2b2_d[1:2, :])
    validT_sb = const.tile([P, t_tiles], f32)
    nc.sync.dma_start(out=validT_sb[:], in_=validT_d[:])

    # persistent activations
    z_all = persist.tile([P, t_tiles * J], f32)
    core_all = persist.tile([P, t_tiles * A], f32)
    sig_all = persist.tile([P, t_tiles * A], f32)
    ep_all = persist.tile([P, t_tiles * A], f32)
    sq_all = persist.tile([P, t_tiles * A], f32)

    # all gather indices resident up-front so the Pool queue never waits on Sync
    idx_all = persist.tile([P, t_tiles * m], i32)
    nc.sync.dma_start(
        out=idx_all[:].rearrange("p (t c) -> p t c", c=m),
        in_=idx_d[:].rearrange("(t p) c -> p t c", p=P),
    )

    # ---------------- phase 1: message passing + linear + BN1 stats ----------
    es1 = ExitStack()
    ph1 = es1.enter_context(tc.tile_pool(name="ph1", bufs=2))
    phg = es1.enter_context(tc.tile_pool(name="phg", bufs=3))
    psum1 = es1.enter_context(tc.tile_pool(name="psum1", bufs=2, space="PSUM"))
    psum_acc = es1.enter_context(tc.tile_pool(name="psum_acc", bufs=1, space="PSUM"))

    statz = psum_acc.tile([1, J], f32, name="statz")
    statz2 = psum_acc.tile([1, J], f32, name="statz2")

    for t in range(t_tiles):
        r0 = t * P
        # loads
        bwi_sb = ph1.tile([P, m], f32, name="bwi_sb")
        nc.sync.dma_start(out=bwi_sb[:], in_=bwi_d[r0 : r0 + P, :])
        bwj_sb = ph1.tile([P, m], f32, name="bwj_sb")
        nc.sync.dma_start(out=bwj_sb[:], in_=bwj_d[r0 : r0 + P, :])
        aself_sb = ph1.tile([P, A], f32, name="aself_sb")
        nc.sync.dma_start(out=aself_sb[:], in_=aself_d[r0 : r0 + P, :])
        nbr_sb = ph1.tile([P, m * B], f32, name="nbr_sb")
        nc.sync.dma_start(out=nbr_sb[:], in_=nbr_d[r0 : r0 + P, :])

        # gather: G[p, c, :] = atab[idx[p, c], :]   (HW: one offset per partition
        # per indirect DMA, so one call per neighbor column)
        G = phg.tile([P, m * A], f32, name="G")
        Gv = G[:].rearrange("p (c e) -> p c e", e=A)
        for c in range(m):
            nc.gpsimd.indirect_dma_start(
                out=Gv[:, c, :],
                out_offset=None,
                in_=atab_d[:],
                in_offset=bass.IndirectOffsetOnAxis(
                    ap=idx_all[:, t * m + c : t * m + c + 1], axis=0
                ),
            )

        # w = bwi * bwj ; s = sum_m w
        w_sb = ph1.tile([P, m], f32, name="w_sb")
        nc.vector.tensor_tensor(out=w_sb[:], in0=bwi_sb[:], in1=bwj_sb[:], op=mybir.AluOpType.mult)
        s_sb = ph1.tile([P, 1], f32, name="s_sb")
        nc.vector.reduce_sum(out=s_sb[:], in_=w_sb[:], axis=mybir.AxisListType.X)

        tg = ph1.tile([P, F], f32, name="tg")

        # self part: tg[:, :A] = aself * s   (ACT per-partition scale)
        nc.scalar.mul(tg[:, 0:A], aself_sb[:], s_sb[:, 0:1])

        # neighbor part: prod[p, g, a] = G[p, g, a] * w[p, g] ; sum over g (24)
        prod = ph1.tile([P, m * A], f32, name="prod")
        nc.vector.tensor_tensor(
            out=prod[:],
            in0=G[:],
            in1=w_sb[:].unsqueeze(2).to_broadcast([P, m, A]),
            op=mybir.AluOpType.mult,
        )
        # reduce tree over g: 24 -> 12 -> 6 -> 3 -> 1
        src = prod[:].rearrange("p (g a) -> p g a", a=A)
        g_cnt = m
        lvl_i = 0
        while g_cnt > 3:
            half = g_cnt // 2
            nxt = ph1.tile([P, half * A], f32, name=f"nlvl{lvl_i}", tag=f"nlvl{lvl_i}")
            nc.vector.tensor_tensor(
                out=nxt[:], in0=src[:, 0:half, :], in1=src[:, half : 2 * half, :],
                op=mybir.AluOpType.add,
            )
            src = nxt[:].rearrange("p (g a) -> p g a", a=A)
            g_cnt = half
            lvl_i += 1
        assert g_cnt == 3
        nl = ph1.tile([P, A], f32, name="nl")
        nc.vector.tensor_tensor(out=nl[:], in0=src[:, 0, :], in1=src[:, 1, :], op=mybir.AluOpType.add)
        nc.vector.tensor_tensor(out=tg[:, A : 2 * A], in0=nl[:], in1=src[:, 2, :], op=mybir.AluOpType.add)

        # bond part: bprod[p, mm, b] = nbr[p, mm, b] * w[p, mm]; sum over mm (24)
        bprod = ph1.tile([P, m * B], f32, name="bprod")
        nc.vector.tensor_tensor(
            out=bprod[:],
            in0=nbr_sb[:],
            in1=w_sb[:].unsqueeze(2).to_broadcast([P, m, B]),
            op=mybir.AluOpType.mult,
        )
        bsrc = bprod[:].rearrange("p (g b) -> p g b", b=B)
        g_cnt = m
        lvl_i = 0
        while g_cnt > 3:
            half = g_cnt // 2
            nxt = ph1.tile([P, half * B], f32, name=f"blvl{lvl_i}", tag=f"blvl{lvl_i}")
            nc.vector.tensor_tensor(
                out=nxt[:], in0=bsrc[:, 0:half, :], in1=bsrc[:, half : 2 * half, :],
                op=mybir.AluOpType.add,
            )
            bsrc = nxt[:].rearrange("p (g b) -> p g b", b=B)
            g_cnt = half
            lvl_i += 1
        assert g_cnt == 3
        bl = ph1.tile([P, B], f32, name="bl")
        nc.vector.tensor_tensor(out=bl[:], in0=bsrc[:, 0, :], in1=bsrc[:, 1, :], op=mybir.AluOpType.add)
        nc.vector.tensor_tensor(out=tg[:, 2 * A : F], in0=bl[:], in1=bsrc[:, 2, :], op=mybir.AluOpType.add)

        # transpose tg -> tgT (two chunks), then z = tg @ W^T
        pT1 = psum1.tile([P, P], f32, name="pT1")
        nc.tensor.transpose(out=pT1[:], in_=tg[:, 0:J], identity=ident[:])
        pT2 = psum1.tile([B, P], f32, name="pT2")
        nc.tensor.transpose(out=pT2[:], in_=tg[:, J:F], identity=ident[:])
        tgT1 = ph1.tile([P, P], f32, name="tgT1")
        nc.vector.tensor_copy(out=tgT1[:], in_=pT1[:])
        tgT2 = ph1.tile([B, P], f32, name="tgT2")
        nc.vector.tensor_copy(out=tgT2[:], in_=pT2[:])

        zp = psum1.tile([P, J], f32, name="zp")
        nc.tensor.matmul(out=zp[:], lhsT=tgT1[:], rhs=wt1_sb[:], start=True, stop=False)
        nc.tensor.matmul(out=zp[:], lhsT=tgT2[:], rhs=wt2_sb[:], start=False, stop=True)

        z_sl = z_all[:, t * J : (t + 1) * J]
        nc.vector.tensor_copy(out=z_sl, in_=zp[:])
        z2_sb = ph1.tile([P, J], f32, name="z2_sb")
        nc.scalar.square(z2_sb[:], zp[:])

        # BN1 partial stats (masked partition sums, accumulated in PSUM)
        vcol = validT_sb[:, t : t + 1]
        nc.tensor.matmul(
            out=statz[:], lhsT=vcol, rhs=z_sl, start=(t == 0), stop=(t == t_tiles - 1),
            skip_group_check=True,
        )
        nc.tensor.matmul(
            out=statz2[:], lhsT=vcol, rhs=z2_sb[:], start=(t == 0), stop=(t == t_tiles - 1),
            skip_group_check=True,
        )

    # ---- AllReduce BN1 stats ----
    sz_sb = persist.tile([1, J], f32)
    nc.vector.tensor_copy(out=sz_sb[:], in_=statz[:])
    sz2_sb = persist.tile([1, J], f32)
    nc.vector.tensor_copy(out=sz2_sb[:], in_=statz2[:])

    dram = es_main.enter_context(tc.tile_pool(name="dram", bufs=1, space="DRAM"))
    ar1_in = dram.tile([2, J], f32)
    ar1_out = dram.tile([2, J], f32, addr_space="Shared")
    nc.sync.dma_start(out=ar1_in[0:1, :], in_=sz_sb[:])
    nc.sync.dma_start(out=ar1_in[1:2, :], in_=sz2_sb[:])
    nc.gpsimd.collective_compute(
        "AllReduce", mybir.AluOpType.add, replica_groups=rg,
        ins=[ar1_in[:].opt()], outs=[ar1_out[:].opt()],
    )
    sum1g = persist.tile([1, J], f32)
    nc.sync.dma_start(out=sum1g[:], in_=ar1_out[0:1, :])
    sq1g = persist.tile([1, J], f32)
    nc.sync.dma_start(out=sq1g[:], in_=ar1_out[1:2, :])

    # ---- BN1 affine coefficients + broadcast ----
    es1.close()

    coef = es_main.enter_context(tc.tile_pool(name="coef", bufs=1))
    psum_b = es_main.enter_context(tc.tile_pool(name="psum_b", bufs=1, space="PSUM"))

    inv_n = 1.0 / float(n)
    mean1 = coef.tile([1, J], f32)
    nc.scalar.mul(mean1[:], sum1g[:], inv_n)
    ex2 = coef.tile([1, J], f32)
    nc.scalar.mul(ex2[:], sq1g[:], inv_n)
    msq = coef.tile([1, J], f32)
    nc.vector.tensor_tensor(out=msq[:], in0=mean1[:], in1=mean1[:], op=mybir.AluOpType.mult)
    var1 = coef.tile([1, J], f32)
    nc.vector.tensor_tensor(out=var1[:], in0=ex2[:], in1=msq[:], op=mybir.AluOpType.subtract)
    nc.vector.tensor_scalar_add(var1[:], var1[:], EPS)
    lnv1 = coef.tile([1, J], f32)
    nc.scalar.activation(lnv1[:], var1[:], mybir.ActivationFunctionType.Ln)
    rstd1 = coef.tile([1, J], f32)
    nc.scalar.activation(rstd1[:], lnv1[:], mybir.ActivationFunctionType.Exp, scale=-0.5)
    # a1 = gamma1 * rstd ; c1 = beta1 - mean * a1  (packed [1, 2J])
    a1c1 = coef.tile([1, 2 * J], f32)
    nc.vector.tensor_tensor(out=a1c1[:, 0:J], in0=gamma1_sb[:], in1=rstd1[:], op=mybir.AluOpType.mult)
    ma1 = coef.tile([1, J], f32)
    nc.vector.tensor_tensor(out=ma1[:], in0=mean1[:], in1=a1c1[:, 0:J], op=mybir.AluOpType.mult)
    nc.vector.tensor_tensor(out=a1c1[:, J : 2 * J], in0=beta1_sb[:], in1=ma1[:], op=mybir.AluOpType.subtract)

    bc1p = psum_b.tile([P, 2 * J], f32)
    nc.tensor.matmul(out=bc1p[:], lhsT=ones1[:], rhs=a1c1[:], start=True, stop=True)
    A1C1 = persist.tile([P, 2 * J], f32)
    nc.vector.tensor_copy(out=A1C1[:], in_=bc1p[:])

    # ---------------- phase 2: BN1 apply + gating + BN2 stats ----------------
    es2 = ExitStack()
    ph2 = es2.enter_context(tc.tile_pool(name="ph2", bufs=2))
    psum2 = es2.enter_context(tc.tile_pool(name="psum2", bufs=1, space="PSUM"))
    statc = psum2.tile([1, A], f32, name="statc")
    statc2 = psum2.tile([1, A], f32, name="statc2")

    # phase 2 as whole-core mega-ops (one instruction per step over all tiles)
    znv = z_all[:].rearrange("p (t j) -> p t j", j=J)
    a1_per = A1C1[:, 0:J].unsqueeze(1).to_broadcast([P, t_tiles, J])
    c1_per = A1C1[:, J : 2 * J].unsqueeze(1).to_broadcast([P, t_tiles, J])
    nc.vector.tensor_tensor(out=znv, in0=znv, in1=a1_per, op=mybir.AluOpType.mult)
    nc.vector.tensor_tensor(out=znv, in0=znv, in1=c1_per, op=mybir.AluOpType.add)
    # sigmoid(zn1) = 1/(1+exp(-zn1)); softplus(zn2) = ln(1+exp(zn2))
    nc.scalar.activation(
        sig_all[:].rearrange("p (t a) -> p t a", a=A),
        znv[:, :, 0:A],
        mybir.ActivationFunctionType.Exp,
        scale=-1.0,
    )
    nc.vector.tensor_scalar_add(sig_all[:], sig_all[:], 1.0)
    nc.vector.reciprocal(sig_all[:], sig_all[:])
    nc.scalar.activation(
        ep_all[:].rearrange("p (t a) -> p t a", a=A),
        znv[:, :, A:J],
        mybir.ActivationFunctionType.Exp,
    )
    nc.scalar.activation(
        ep_all[:], ep_all[:], mybir.ActivationFunctionType.Ln, bias=1.0
    )
    # core = sig * softplus, masked by row validity for the stats
    nc.vector.tensor_tensor(out=core_all[:], in0=sig_all[:], in1=ep_all[:], op=mybir.AluOpType.mult)
    vmask = validT_sb[:].unsqueeze(2).to_broadcast([P, t_tiles, A])
    nc.vector.tensor_tensor(
        out=core_all[:].rearrange("p (t a) -> p t a", a=A),
        in0=core_all[:].rearrange("p (t a) -> p t a", a=A),
        in1=vmask,
        op=mybir.AluOpType.mult,
    )
    nc.scalar.square(sq_all[:], core_all[:])
    # per-partition sums over t, then partition-sum via ones-matmul
    ms_c = coef2 = None
    ms_c = persist.tile([P, A], f32)
    nc.vector.reduce_sum(
        out=ms_c[:],
        in_=core_all[:].rearrange("p (t a) -> p a t", a=A),
        axis=mybir.AxisListType.X,
    )
    ms_c2 = persist.tile([P, A], f32)
    nc.vector.reduce_sum(
        out=ms_c2[:],
        in_=sq_all[:].rearrange("p (t a) -> p a t", a=A),
        axis=mybir.AxisListType.X,
    )
    ones128 = const.tile([P, 1], f32)
    nc.vector.memset(ones128[:], 1.0)
    statc = psum2.tile([1, A], f32, name="statc")
    statc2 = psum2.tile([1, A], f32, name="statc2")
    nc.tensor.matmul(out=statc[:], lhsT=ones128[:], rhs=ms_c[:], start=True, stop=True)
    nc.tensor.matmul(out=statc2[:], lhsT=ones128[:], rhs=ms_c2[:], start=True, stop=True)

    # ---- AllReduce BN2 stats ----
    sc_sb = persist.tile([1, A], f32)
    nc.vector.tensor_copy(out=sc_sb[:], in_=statc[:])
    sc2_sb = persist.tile([1, A], f32)
    nc.vector.tensor_copy(out=sc2_sb[:], in_=statc2[:])
    ar2_in = dram.tile([2, A], f32)
    ar2_out = dram.tile([2, A], f32, addr_space="Shared")
    nc.sync.dma_start(out=ar2_in[0:1, :], in_=sc_sb[:])
    nc.sync.dma_start(out=ar2_in[1:2, :], in_=sc2_sb[:])
    nc.gpsimd.collective_compute(
        "AllReduce", mybir.AluOpType.add, replica_groups=rg,
        ins=[ar2_in[:].opt()], outs=[ar2_out[:].opt()],
    )
    sum2g = persist.tile([1, A], f32)
    nc.sync.dma_start(out=sum2g[:], in_=ar2_out[0:1, :])
    sq2g = persist.tile([1, A], f32)
    nc.sync.dma_start(out=sq2g[:], in_=ar2_out[1:2, :])

    mean2 = coef.tile([1, A], f32)
    nc.scalar.mul(mean2[:], sum2g[:], inv_n)
    ex22 = coef.tile([1, A], f32)
    nc.scalar.mul(ex22[:], sq2g[:], inv_n)
    msq2 = coef.tile([1, A], f32)
    nc.vector.tensor_tensor(out=msq2[:], in0=mean2[:], in1=mean2[:], op=mybir.AluOpType.mult)
    var2 = coef.tile([1, A], f32)
    nc.vector.tensor_tensor(out=var2[:], in0=ex22[:], in1=msq2[:], op=mybir.AluOpType.subtract)
    nc.vector.tensor_scalar_add(var2[:], var2[:], EPS)
    lnv2 = coef.tile([1, A], f32)
    nc.scalar.activation(lnv2[:], var2[:], mybir.ActivationFunctionType.Ln)
    rstd2 = coef.tile([1, A], f32)
    nc.scalar.activation(rstd2[:], lnv2[:], mybir.ActivationFunctionType.Exp, scale=-0.5)
    a2c2 = coef.tile([1, 2 * A], f32)
    nc.vector.tensor_tensor(out=a2c2[:, 0:A], in0=gamma2_sb[:], in1=rstd2[:], op=mybir.AluOpType.mult)
    ma2 = coef.tile([1, A], f32)
    nc.vector.tensor_tensor(out=ma2[:], in0=mean2[:], in1=a2c2[:, 0:A], op=mybir.AluOpType.mult)
    nc.vector.tensor_tensor(out=a2c2[:, A : 2 * A], in0=beta2_sb[:], in1=ma2[:], op=mybir.AluOpType.subtract)

    bc2p = psum_b.tile([P, 2 * A], f32)
    nc.tensor.matmul(out=bc2p[:], lhsT=ones1[:], rhs=a2c2[:], start=True, stop=True)
    A2C2 = persist.tile([P, 2 * A], f32)
    nc.vector.tensor_copy(out=A2C2[:], in_=bc2p[:])

    # ---------------- phase 3: BN2 apply + softplus + store ----------------
    corev = core_all[:].rearrange("p (t a) -> p t a", a=A)
    a2_per = A2C2[:, 0:A].unsqueeze(1).to_broadcast([P, t_tiles, A])
    c2_per = A2C2[:, A : 2 * A].unsqueeze(1).to_broadcast([P, t_tiles, A])
    nc.vector.tensor_tensor(out=corev, in0=corev, in1=a2_per, op=mybir.AluOpType.mult)
    nc.vector.tensor_tensor(out=corev, in0=corev, in1=c2_per, op=mybir.AluOpType.add)
    nc.scalar.activation(core_all[:], core_all[:], mybir.ActivationFunctionType.Exp)
    nc.scalar.activation(core_all[:], core_all[:], mybir.ActivationFunctionType.Ln, bias=1.0)
    nc.sync.dma_start(
        out=out_d[:].rearrange("(t p) a -> p t a", p=P),
        in_=corev,
    )

    es2.close()
    es_main.close()
    tc.__exit__(None, None, None)
    nc_.compile()
    return nc_


def make_in_maps(inputs, n=N, m=M, cores=CORES):
    """Host-side sharding/layout prep (index reshuffling + dtype conversion only)."""
    ns = n // cores
    t_tiles = (ns + P - 1) // P
    npad = t_tiles * P

    atom = np.asarray(inputs["atom_in_fea"], np.float32)
    nbr = np.asarray(inputs["nbr_fea"], np.float32).reshape(n, m * B)
    idx = np.asarray(inputs["nbr_fea_idx"])
    bwi = np.asarray(inputs["bond_weights_i"], np.float32)
    bwj = np.asarray(inputs["bond_weights_j"], np.float32)
    W = np.asarray(inputs["W"], np.float32)
    g1 = np.asarray(inputs["gamma1"], np.float32)
    b1 = np.asarray(inputs["beta1"], np.float32)
    g2 = np.asarray(inputs["gamma2"], np.float32)
    b2 = np.asarray(inputs["beta2"], np.float32)

    Wt = np.ascontiguousarray(W.T)  # [F, J]
    wt1 = np.ascontiguousarray(Wt[0:J, :])
    wt2 = np.ascontiguousarray(Wt[J:F, :])
    g1b1 = np.stack([g1, b1]).astype(np.float32)
    g2b2 = np.stack([g2, b2]).astype(np.float32)

    valid = np.zeros((npad,), np.float32)
    valid[:ns] = 1.0
    validT = np.ascontiguousarray(valid.reshape(t_tiles, P).T)

    in_maps = []
    for c in range(cores):
        lo, hi = c * ns, (c + 1) * ns
        pad = npad - ns

        def padrows(x):
            return np.concatenate([x, np.zeros((pad,) + x.shape[1:], x.dtype)], 0) if pad else x

        idx_c = np.concatenate([idx[lo:hi], np.zeros((pad, m), idx.dtype)], 0) if pad else idx[lo:hi]

        in_maps.append(
            {
                "atab": atom,
                "aself": padrows(atom[lo:hi]),
                "nbr": padrows(nbr[lo:hi]),
                "bwi": padrows(bwi[lo:hi]),
                "bwj": padrows(bwj[lo:hi]),
                "idx": np.ascontiguousarray(idx_c.astype(np.int32)),
                "validT": validT,
                "wt1": wt1,
                "wt2": wt2,
                "g1b1": g1b1,
                "g2b2": g2b2,
                "ident": np.eye(P, dtype=np.float32),
            }
        )
    return in_maps


_GRAPH_CACHE = {}


def _get_graph():
    if "nc" not in _GRAPH_CACHE:
        _GRAPH_CACHE["nc"] = build_graph()
    return _GRAPH_CACHE["nc"]


def run(inputs, trace=False, **kw):
    nc = _get_graph()
    in_maps = make_in_maps(inputs)
    res = run_bass_kernel_spmd(nc, in_maps, core_ids=list(range(CORES)), trace=trace, **kw)
    ns = N // CORES
    out = np.concatenate([res.results[c]["out"][:ns] for c in range(CORES)], 0)
    return out.astype(np.float32), res


def kernel(**inputs) -> np.ndarray:
    out, _ = run(inputs, trace=False)
    return out


# revision 14
# speedup vs baseline: 1.3571x; 1.0113x over previous
"""AtomConvLayer (CGCNN message passing) distributed Bass kernel for 8 TRN2 NeuronCores.

Strategy (data-parallel over atoms N):
  - Each core owns N/8 = 6250 atom rows (padded to 6272 = 49*128); the atom
    feature table is replicated to every core for the neighbor gather.
  - The gather atom_in_fea[nbr_fea_idx] runs on-device via indirect DMA
    (`indirect_dma_start`, one offset per partition -> 128 rows/call; 24 calls
    per 128-row tile). This image lacks the custom GPSIMD ucode (dma_gather),
    so the Q7 SWDGE descriptor loop is the kernel's critical path.
  - Per 128-row tile: w = bwi*bwj; self/nbr/bond parts reduced on DVE with
    broadcast-AP multiplies and contiguous add-trees; concat -> PE transpose ->
    matmul with W^T -> z kept resident in SBUF.
  - BatchNorm stats are partition-reduced per tile with a valid-mask matmul
    accumulated in PSUM, then AllReduce'd across the 8 cores (sum|sumsq packed).
  - BN1 affine + sigmoid*softplus and BN2 + softplus run as whole-core mega-ops
    (single instructions over all 49 tiles with periodic broadcast APs) to keep
    the post-collective tail short; the linear-layer bias is dropped since
    BatchNorm cancels it.
"""
import sys

sys.path.insert(0, "/opt/trn_rl_repo")

import numpy as np

from concourse import bass, bacc, mybir, tile
from concourse.bass_utils import run_bass_kernel_spmd

# problem sizes (hardcoded per spec)
N = 50000
M = 24
A = 64  # atom_fea_len
B = 32  # nbr_fea_len
F = 2 * A + B  # 160
J = 2 * A  # 128
EPS = 1e-5

CORES = 8
P = 128
NS = N // CORES  # 6250
T = (NS + P - 1) // P  # 49 tiles
NPAD = T * P  # 6272

f32 = mybir.dt.float32
i32 = mybir.dt.int32


def build_graph(n=N, m=M, cores=CORES):
    """Build the SPMD Tile graph. Parameterized so a scaled-down version can be
    simulated; the real kernel uses the module constants."""
    ns = n // cores
    t_tiles = (ns + P - 1) // P
    npad = t_tiles * P

    nc_ = bacc.Bacc("TRN2", target_bir_lowering=False, debug=False, num_devices=cores)
    tc = tile.TileContext(nc_)
    tc.__enter__()
    nc = tc.nc

    # ---- DRAM parameters (per-core shards supplied via in_maps) ----
    atab_d = nc.dram_tensor("atab", [n, A], f32, kind="ExternalInput")
    aself_d = nc.dram_tensor("aself", [npad, A], f32, kind="ExternalInput")
    nbr_d = nc.dram_tensor("nbr", [npad, m * B], f32, kind="ExternalInput")
    bwi_d = nc.dram_tensor("bwi", [npad, m], f32, kind="ExternalInput")
    bwj_d = nc.dram_tensor("bwj", [npad, m], f32, kind="ExternalInput")
    idx_d = nc.dram_tensor("idx", [npad, m], i32, kind="ExternalInput")
    validT_d = nc.dram_tensor("validT", [P, t_tiles], f32, kind="ExternalInput")
    wt1_d = nc.dram_tensor("wt1", [J, J], f32, kind="ExternalInput")
    wt2_d = nc.dram_tensor("wt2", [B, J], f32, kind="ExternalInput")
    g1b1_d = nc.dram_tensor("g1b1", [2, J], f32, kind="ExternalInput")
    g2b2_d = nc.dram_tensor("g2b2", [2, A], f32, kind="ExternalInput")
    ident_d = nc.dram_tensor("ident", [P, P], f32, kind="ExternalInput")
    out_d = nc.dram_tensor("out", [npad, A], f32, kind="ExternalOutput")

    rg = [list(range(cores))]

    from contextlib import ExitStack

    es_main = ExitStack()
    const = es_main.enter_context(tc.tile_pool(name="const", bufs=1))
    persist = es_main.enter_context(tc.tile_pool(name="persist", bufs=1))

    # constants
    ident = const.tile([P, P], f32)
    nc.sync.dma_start(out=ident[:], in_=ident_d[:])
    ones1 = const.tile([1, P], f32)
    nc.vector.memset(ones1[:], 1.0)
    wt1_sb = const.tile([J, J], f32)
    nc.sync.dma_start(out=wt1_sb[:], in_=wt1_d[:])
    wt2_sb = const.tile([B, J], f32)
    nc.sync.dma_start(out=wt2_sb[:], in_=wt2_d[:])
    gamma1_sb = const.tile([1, J], f32)
    nc.sync.dma_start(out=gamma1_sb[:], in_=g1b1_d[0:1, :])
    beta1_sb = const.tile([1, J], f32)
    nc.sync.dma_start(out=beta1_sb[:], in_=g1b1_d[1:2, :])
    gamma2_sb = const.tile([1, A], f32)
    nc.sync.dma_start(out=gamma2_sb[:], in_=g2b2_d[0:1, :])
    beta2_sb = const.tile([1, A], f32)
    nc.sync.dma_start(out=beta2_sb[:], in_=g2b2_d[1:2, :])
    validT_sb = const.tile([P, t_tiles], f32)
    nc.sync.dma_start(out=validT_sb[:], in_=validT_d[:])

    # persistent activations
    z_all = persist.tile([P, t_tiles * J], f32)
    core_all = persist.tile([P, t_tiles * A], f32)
    sig_all = persist.tile([P, t_tiles * A], f32)
    ep_all = persist.tile([P, t_tiles * A], f32)
    sq_all = persist.tile([P, t_tiles * A], f32)

    # all gather indices resident up-front so the Pool queue never waits on Sync
    idx_all = persist.tile([P, t_tiles * m], i32)
    nc.sync.dma_start(
        out=idx_all[:].rearrange("p (t c) -> p t c", c=m),
        in_=idx_d[:].rearrange("(t p) c -> p t c", p=P),
    )

    # ---------------- phase 1: message passing + linear + BN1 stats ----------
    es1 = ExitStack()
    ph1 = es1.enter_context(tc.tile_pool(name="ph1", bufs=2))
    phg = es1.enter_context(tc.tile_pool(name="phg", bufs=3))
    psum1 = es1.enter_context(tc.tile_pool(name="psum1", bufs=2, space="PSUM"))
    psum_acc = es1.enter_context(tc.tile_pool(name="psum_acc", bufs=1, space="PSUM"))

    statz = psum_acc.tile([1, J], f32, name="statz")
    statz2 = psum_acc.tile([1, J], f32, name="statz2")

    for t in range(t_tiles):
        r0 = t * P
        # loads
        bwi_sb = ph1.tile([P, m], f32, name="bwi_sb")
        nc.sync.dma_start(out=bwi_sb[:], in_=bwi_d[r0 : r0 + P, :])
        bwj_sb = ph1.tile([P, m], f32, name="bwj_sb")
        nc.sync.dma_start(out=bwj_sb[:], in_=bwj_d[r0 : r0 + P, :])
        aself_sb = ph1.tile([P, A], f32, name="aself_sb")
        nc.sync.dma_start(out=aself_sb[:], in_=aself_d[r0 : r0 + P, :])
        nbr_sb = ph1.tile([P, m * B], f32, name="nbr_sb")
        nc.sync.dma_start(out=nbr_sb[:], in_=nbr_d[r0 : r0 + P, :])

        # gather: G[p, c, :] = atab[idx[p, c], :]   (HW: one offset per partition
        # per indirect DMA, so one call per neighbor column)
        G = phg.tile([P, m * A], f32, name="G")
        Gv = G[:].rearrange("p (c e) -> p c e", e=A)
        for c in range(m):
            nc.gpsimd.indirect_dma_start(
                out=Gv[:, c, :],
                out_offset=None,
                in_=atab_d[:],
                in_offset=bass.IndirectOffsetOnAxis(
                    ap=idx_all[:, t * m + c : t * m + c + 1], axis=0
                ),
            )

        # w = bwi * bwj ; s = sum_m w
        w_sb = ph1.tile([P, m], f32, name="w_sb")
        nc.vector.tensor_tensor(out=w_sb[:], in0=bwi_sb[:], in1=bwj_sb[:], op=mybir.AluOpType.mult)
        s_sb = ph1.tile([P, 1], f32, name="s_sb")
        nc.vector.reduce_sum(out=s_sb[:], in_=w_sb[:], axis=mybir.AxisListType.X)

        tg = ph1.tile([P, F], f32, name="tg")

        # self part: tg[:, :A] = aself * s   (ACT per-partition scale)
        nc.scalar.mul(tg[:, 0:A], aself_sb[:], s_sb[:, 0:1])

        # neighbor part: prod[p, g, a] = G[p, g, a] * w[p, g] ; sum over g (24)
        prod = ph1.tile([P, m * A], f32, name="prod")
        nc.vector.tensor_tensor(
            out=prod[:],
            in0=G[:],
            in1=w_sb[:].unsqueeze(2).to_broadcast([P, m, A]),
            op=mybir.AluOpType.mult,
        )
        # reduce tree over g: 24 -> 12 -> 6 -> 3 -> 1
        src = prod[:].rearrange("p (g a) -> p g a", a=A)
        g_cnt = m
        lvl_i = 0
        while g_cnt > 3:
            half = g_cnt // 2
            nxt = ph1.tile([P, half * A], f32, name=f"nlvl{lvl_i}", tag=f"nlvl{lvl_i}")
            nc.vector.tensor_tensor(
                out=nxt[:], in0=src[:, 0:half, :], in1=src[:, half : 2 * half, :],
                op=mybir.AluOpType.add,
            )
            src = nxt[:].rearrange("p (g a) -> p g a", a=A)
            g_cnt = half
            lvl_i += 1
        assert g_cnt == 3
        nl = ph1.tile([P, A], f32, name="nl")
        nc.vector.tensor_tensor(out=nl[:], in0=src[:, 0, :], in1=src[:, 1, :], op=mybir.AluOpType.add)
        nc.vector.tensor_tensor(out=tg[:, A : 2 * A], in0=nl[:], in1=src[:, 2, :], op=mybir.AluOpType.add)

        # bond part: bprod[p, mm, b] = nbr[p, mm, b] * w[p, mm]; sum over mm (24)
        bprod = ph1.tile([P, m * B], f32, name="bprod")
        nc.vector.tensor_tensor(
            out=bprod[:],
            in0=nbr_sb[:],
            in1=w_sb[:].unsqueeze(2).to_broadcast([P, m, B]),
            op=mybir.AluOpType.mult,
        )
        bsrc = bprod[:].rearrange("p (g b) -> p g b", b=B)
        g_cnt = m
        lvl_i = 0
        while g_cnt > 3:
            half = g_cnt // 2
            nxt = ph1.tile([P, half * B], f32, name=f"blvl{lvl_i}", tag=f"blvl{lvl_i}")
            nc.vector.tensor_tensor(
                out=nxt[:], in0=bsrc[:, 0:half, :], in1=bsrc[:, half : 2 * half, :],
                op=mybir.AluOpType.add,
            )
            bsrc = nxt[:].rearrange("p (g b) -> p g b", b=B)
            g_cnt = half
            lvl_i += 1
        assert g_cnt == 3
        bl = ph1.tile([P, B], f32, name="bl")
        nc.vector.tensor_tensor(out=bl[:], in0=bsrc[:, 0, :], in1=bsrc[:, 1, :], op=mybir.AluOpType.add)
        nc.vector.tensor_tensor(out=tg[:, 2 * A : F], in0=bl[:], in1=bsrc[:, 2, :], op=mybir.AluOpType.add)

        # transpose tg -> tgT (two chunks), then z = tg @ W^T
        pT1 = psum1.tile([P, P], f32, name="pT1")
        nc.tensor.transpose(out=pT1[:], in_=tg[:, 0:J], identity=ident[:])
        pT2 = psum1.tile([B, P], f32, name="pT2")
        nc.tensor.transpose(out=pT2[:], in_=tg[:, J:F], identity=ident[:])
        tgT1 = ph1.tile([P, P], f32, name="tgT1")
        nc.vector.tensor_copy(out=tgT1[:], in_=pT1[:])
        tgT2 = ph1.tile([B, P], f32, name="tgT2")
        nc.vector.tensor_copy(out=tgT2[:], in_=pT2[:])

        zp = psum1.tile([P, J], f32, name="zp")
        nc.tensor.matmul(out=zp[:], lhsT=tgT1[:], rhs=wt1_sb[:], start=True, stop=False)
        nc.tensor.matmul(out=zp[:], lhsT=tgT2[:], rhs=wt2_sb[:], start=False, stop=True)

        z_sl = z_all[:, t * J : (t + 1) * J]
        nc.vector.tensor_copy(out=z_sl, in_=zp[:])
        z2_sb = ph1.tile([P, J], f32, name="z2_sb")
        nc.scalar.square(z2_sb[:], zp[:])

        # BN1 partial stats (masked partition sums, accumulated in PSUM)
        vcol = validT_sb[:, t : t + 1]
        nc.tensor.matmul(
            out=statz[:], lhsT=vcol, rhs=z_sl, start=(t == 0), stop=(t == t_tiles - 1),
            skip_group_check=True,
        )
        nc.tensor.matmul(
            out=statz2[:], lhsT=vcol, rhs=z2_sb[:], start=(t == 0), stop=(t == t_tiles - 1),
            skip_group_check=True,
        )

    # ---- AllReduce BN1 stats ----
    s1_pack = persist.tile([1, 2 * J], f32)
    nc.vector.tensor_copy(out=s1_pack[:, 0:J], in_=statz[:])
    nc.vector.tensor_copy(out=s1_pack[:, J : 2 * J], in_=statz2[:])

    dram = es_main.enter_context(tc.tile_pool(name="dram", bufs=1, space="DRAM"))
    ar1_in = dram.tile([1, 2 * J], f32)
    ar1_out = dram.tile([1, 2 * J], f32, addr_space="Shared")
    nc.sync.dma_start(out=ar1_in[:], in_=s1_pack[:])
    nc.gpsimd.collective_compute(
        "AllReduce", mybir.AluOpType.add, replica_groups=rg,
        ins=[ar1_in[:].opt()], outs=[ar1_out[:].opt()],
    )
    s1g = persist.tile([1, 2 * J], f32)
    nc.sync.dma_start(out=s1g[:], in_=ar1_out[:])
    sum1g = s1g[:, 0:J]
    sq1g = s1g[:, J : 2 * J]

    # ---- BN1 affine coefficients + broadcast ----
    es1.close()

    coef = es_main.enter_context(tc.tile_pool(name="coef", bufs=1))
    psum_b = es_main.enter_context(tc.tile_pool(name="psum_b", bufs=1, space="PSUM"))

    inv_n = 1.0 / float(n)
    mean1 = coef.tile([1, J], f32)
    nc.scalar.mul(mean1[:], sum1g, inv_n)
    ex2 = coef.tile([1, J], f32)
    nc.scalar.mul(ex2[:], sq1g, inv_n)
    msq = coef.tile([1, J], f32)
    nc.vector.tensor_tensor(out=msq[:], in0=mean1[:], in1=mean1[:], op=mybir.AluOpType.mult)
    var1 = coef.tile([1, J], f32)
    nc.vector.tensor_tensor(out=var1[:], in0=ex2[:], in1=msq[:], op=mybir.AluOpType.subtract)
    nc.vector.tensor_scalar_add(var1[:], var1[:], EPS)
    lnv1 = coef.tile([1, J], f32)
    nc.scalar.activation(lnv1[:], var1[:], mybir.ActivationFunctionType.Ln)
    rstd1 = coef.tile([1, J], f32)
    nc.scalar.activation(rstd1[:], lnv1[:], mybir.ActivationFunctionType.Exp, scale=-0.5)
    # a1 = gamma1 * rstd ; c1 = beta1 - mean * a1  (packed [1, 2J])
    a1c1 = coef.tile([1, 2 * J], f32)
    nc.vector.tensor_tensor(out=a1c1[:, 0:J], in0=gamma1_sb[:], in1=rstd1[:], op=mybir.AluOpType.mult)
    ma1 = coef.tile([1, J], f32)
    nc.vector.tensor_tensor(out=ma1[:], in0=mean1[:], in1=a1c1[:, 0:J], op=mybir.AluOpType.mult)
    nc.vector.tensor_tensor(out=a1c1[:, J : 2 * J], in0=beta1_sb[:], in1=ma1[:], op=mybir.AluOpType.subtract)

    bc1p = psum_b.tile([P, 2 * J], f32)
    nc.tensor.matmul(out=bc1p[:], lhsT=ones1[:], rhs=a1c1[:], start=True, stop=True)
    A1C1 = persist.tile([P, 2 * J], f32)
    nc.vector.tensor_copy(out=A1C1[:], in_=bc1p[:])

    # ---------------- phase 2: BN1 apply + gating + BN2 stats ----------------
    es2 = ExitStack()
    ph2 = es2.enter_context(tc.tile_pool(name="ph2", bufs=2))
    psum2 = es2.enter_context(tc.tile_pool(name="psum2", bufs=1, space="PSUM"))
    statc = psum2.tile([1, A], f32, name="statc")
    statc2 = psum2.tile([1, A], f32, name="statc2")

    # phase 2 as whole-core mega-ops (one instruction per step over all tiles)
    znv = z_all[:].rearrange("p (t j) -> p t j", j=J)
    a1_per = A1C1[:, 0:J].unsqueeze(1).to_broadcast([P, t_tiles, J])
    c1_per = A1C1[:, J : 2 * J].unsqueeze(1).to_broadcast([P, t_tiles, J])
    nc.vector.tensor_tensor(out=znv, in0=znv, in1=a1_per, op=mybir.AluOpType.mult)
    nc.vector.tensor_tensor(out=znv, in0=znv, in1=c1_per, op=mybir.AluOpType.add)
    # sigmoid(zn1) = 1/(1+exp(-zn1)); softplus(zn2) = ln(1+exp(zn2))
    nc.scalar.activation(
        sig_all[:].rearrange("p (t a) -> p t a", a=A),
        znv[:, :, 0:A],
        mybir.ActivationFunctionType.Sigmoid,
    )
    nc.scalar.activation(
        ep_all[:].rearrange("p (t a) -> p t a", a=A),
        znv[:, :, A:J],
        mybir.ActivationFunctionType.Exp,
    )
    nc.scalar.activation(
        ep_all[:], ep_all[:], mybir.ActivationFunctionType.Ln, bias=1.0
    )
    # core = sig * softplus, masked by row validity for the stats
    nc.vector.tensor_tensor(out=core_all[:], in0=sig_all[:], in1=ep_all[:], op=mybir.AluOpType.mult)
    vmask = validT_sb[:].unsqueeze(2).to_broadcast([P, t_tiles, A])
    nc.vector.tensor_tensor(
        out=core_all[:].rearrange("p (t a) -> p t a", a=A),
        in0=core_all[:].rearrange("p (t a) -> p t a", a=A),
        in1=vmask,
        op=mybir.AluOpType.mult,
    )
    nc.scalar.square(sq_all[:], core_all[:])
    # per-partition sums over t, then partition-sum via ones-matmul
    ms_c = coef2 = None
    ms_c = persist.tile([P, A], f32)
    nc.vector.reduce_sum(
        out=ms_c[:],
        in_=core_all[:].rearrange("p (t a) -> p a t", a=A),
        axis=mybir.AxisListType.X,
    )
    ms_c2 = persist.tile([P, A], f32)
    nc.vector.reduce_sum(
        out=ms_c2[:],
        in_=sq_all[:].rearrange("p (t a) -> p a t", a=A),
        axis=mybir.AxisListType.X,
    )
    ones128 = const.tile([P, 1], f32)
    nc.vector.memset(ones128[:], 1.0)
    statc = psum2.tile([1, A], f32, name="statc")
    statc2 = psum2.tile([1, A], f32, name="statc2")
    nc.tensor.matmul(out=statc[:], lhsT=ones128[:], rhs=ms_c[:], start=True, stop=True)
    nc.tensor.matmul(out=statc2[:], lhsT=ones128[:], rhs=ms_c2[:], start=True, stop=True)

    # ---- AllReduce BN2 stats ----
    s2_pack = persist.tile([1, 2 * A], f32)
    nc.vector.tensor_copy(out=s2_pack[:, 0:A], in_=statc[:])
    nc.vector.tensor_copy(out=s2_pack[:, A : 2 * A], in_=statc2[:])
    ar2_in = dram.tile([1, 2 * A], f32)
    ar2_out = dram.tile([1, 2 * A], f32, addr_space="Shared")
    nc.sync.dma_start(out=ar2_in[:], in_=s2_pack[:])
    nc.gpsimd.collective_compute(
        "AllReduce", mybir.AluOpType.add, replica_groups=rg,
        ins=[ar2_in[:].opt()], outs=[ar2_out[:].opt()],
    )
    s2g = persist.tile([1, 2 * A], f32)
    nc.sync.dma_start(out=s2g[:], in_=ar2_out[:])
    sum2g_ap = s2g[:, 0:A]
    sq2g_ap = s2g[:, A : 2 * A]

    mean2 = coef.tile([1, A], f32)
    nc.scalar.mul(mean2[:], sum2g_ap, inv_n)
    ex22 = coef.tile([1, A], f32)
    nc.scalar.mul(ex22[:], sq2g_ap, inv_n)
    msq2 = coef.tile([1, A], f32)
    nc.vector.tensor_tensor(out=msq2[:], in0=mean2[:], in1=mean2[:], op=mybir.AluOpType.mult)
    var2 = coef.tile([1, A], f32)
    nc.vector.tensor_tensor(out=var2[:], in0=ex22[:], in1=msq2[:], op=mybir.AluOpType.subtract)
    nc.vector.tensor_scalar_add(var2[:], var2[:], EPS)
    lnv2 = coef.tile([1, A], f32)
    nc.scalar.activation(lnv2[:], var2[:], mybir.ActivationFunctionType.Ln)
    rstd2 = coef.tile([1, A], f32)
    nc.scalar.activation(rstd2[:], lnv2[:], mybir.ActivationFunctionType.Exp, scale=-0.5)
    a2c2 = coef.tile([1, 2 * A], f32)
    nc.vector.tensor_tensor(out=a2c2[:, 0:A], in0=gamma2_sb[:], in1=rstd2[:], op=mybir.AluOpType.mult)
    ma2 = coef.tile([1, A], f32)
    nc.vector.tensor_tensor(out=ma2[:], in0=mean2[:], in1=a2c2[:, 0:A], op=mybir.AluOpType.mult)
    nc.vector.tensor_tensor(out=a2c2[:, A : 2 * A], in0=beta2_sb[:], in1=ma2[:], op=mybir.AluOpType.subtract)

    bc2p = psum_b.tile([P, 2 * A], f32)
    nc.tensor.matmul(out=bc2p[:], lhsT=ones1[:], rhs=a2c2[:], start=True, stop=True)
    A2C2 = persist.tile([P, 2 * A], f32)
    nc.vector.tensor_copy(out=A2C2[:], in_=bc2p[:])

    # ---------------- phase 3: BN2 apply + softplus + store ----------------
    corev = core_all[:].rearrange("p (t a) -> p t a", a=A)
    a2_per = A2C2[:, 0:A].unsqueeze(1).to_broadcast([P, t_tiles, A])
    c2_per = A2C2[:, A : 2 * A].unsqueeze(1).to_broadcast([P, t_tiles, A])
    nc.vector.tensor_tensor(out=corev, in0=corev, in1=a2_per, op=mybir.AluOpType.mult)
    nc.vector.tensor_tensor(out=corev, in0=corev, in1=c2_per, op=mybir.AluOpType.add)
    nc.scalar.activation(core_all[:], core_all[:], mybir.ActivationFunctionType.Exp)
    nc.scalar.activation(core_all[:], core_all[:], mybir.ActivationFunctionType.Ln, bias=1.0)
    nc.sync.dma_start(
        out=out_d[:].rearrange("(t p) a -> p t a", p=P),
        in_=corev,
    )

    es2.close()
    es_main.close()
    tc.__exit__(None, None, None)
    nc_.compile()
    return nc_


def make_in_maps(inputs, n=N, m=M, cores=CORES):
    """Host-side sharding/layout prep (index reshuffling + dtype conversion only)."""
    ns = n // cores
    t_tiles = (ns + P - 1) // P
    npad = t_tiles * P

    atom = np.asarray(inputs["atom_in_fea"], np.float32)
    nbr = np.asarray(inputs["nbr_fea"], np.float32).reshape(n, m * B)
    idx = np.asarray(inputs["nbr_fea_idx"])
    bwi = np.asarray(inputs["bond_weights_i"], np.float32)
    bwj = np.asarray(inputs["bond_weights_j"], np.float32)
    W = np.asarray(inputs["W"], np.float32)
    g1 = np.asarray(inputs["gamma1"], np.float32)
    b1 = np.asarray(inputs["beta1"], np.float32)
    g2 = np.asarray(inputs["gamma2"], np.float32)
    b2 = np.asarray(inputs["beta2"], np.float32)

    Wt = np.ascontiguousarray(W.T)  # [F, J]
    wt1 = np.ascontiguousarray(Wt[0:J, :])
    wt2 = np.ascontiguousarray(Wt[J:F, :])
    g1b1 = np.stack([g1, b1]).astype(np.float32)
    g2b2 = np.stack([g2, b2]).astype(np.float32)

    valid = np.zeros((npad,), np.float32)
    valid[:ns] = 1.0
    validT = np.ascontiguousarray(valid.reshape(t_tiles, P).T)

    in_maps = []
    for c in range(cores):
        lo, hi = c * ns, (c + 1) * ns
        pad = npad - ns

        def padrows(x):
            return np.concatenate([x, np.zeros((pad,) + x.shape[1:], x.dtype)], 0) if pad else x

        idx_c = np.concatenate([idx[lo:hi], np.zeros((pad, m), idx.dtype)], 0) if pad else idx[lo:hi]

        in_maps.append(
            {
                "atab": atom,
                "aself": padrows(atom[lo:hi]),
                "nbr": padrows(nbr[lo:hi]),
                "bwi": padrows(bwi[lo:hi]),
                "bwj": padrows(bwj[lo:hi]),
                "idx": np.ascontiguousarray(idx_c.astype(np.int32)),
                "validT": validT,
                "wt1": wt1,
                "wt2": wt2,
                "g1b1": g1b1,
                "g2b2": g2b2,
                "ident": np.eye(P, dtype=np.float32),
            }
        )
    return in_maps


_GRAPH_CACHE = {}


def _get_graph():
    if "nc" not in _GRAPH_CACHE:
        _GRAPH_CACHE["nc"] = build_graph()
    return _GRAPH_CACHE["nc"]


def run(inputs, trace=False, **kw):
    nc = _get_graph()
    in_maps = make_in_maps(inputs)
    res = run_bass_kernel_spmd(nc, in_maps, core_ids=list(range(CORES)), trace=trace, **kw)
    ns = N // CORES
    out = np.concatenate([res.results[c]["out"][:ns] for c in range(CORES)], 0)
    return out.astype(np.float32), res


def kernel(**inputs) -> np.ndarray:
    out, _ = run(inputs, trace=False)
    return out


# revision 15
# speedup vs baseline: 1.3617x; 1.0034x over previous
"""AtomConvLayer (CGCNN message passing) distributed Bass kernel for 8 TRN2 NeuronCores.

Strategy (data-parallel over atoms N):
  - Each core owns N/8 = 6250 atom rows (padded to 6272 = 49*128); the atom
    feature table is replicated to every core for the neighbor gather.
  - The gather atom_in_fea[nbr_fea_idx] runs on-device via indirect DMA
    (`indirect_dma_start`, one offset per partition -> 128 rows/call; 24 calls
    per 128-row tile). This image lacks the custom GPSIMD ucode (dma_gather),
    so the Q7 SWDGE descriptor loop is the kernel's critical path.
  - Per 128-row tile: w = bwi*bwj; self/nbr/bond parts reduced on DVE with
    broadcast-AP multiplies and contiguous add-trees; concat -> PE transpose ->
    matmul with W^T -> z kept resident in SBUF.
  - BatchNorm stats are partition-reduced per tile with a valid-mask matmul
    accumulated in PSUM, then AllReduce'd across the 8 cores (sum|sumsq packed).
  - BN1 affine + sigmoid*softplus and BN2 + softplus run as whole-core mega-ops
    (single instructions over all 49 tiles with periodic broadcast APs) to keep
    the post-collective tail short; the linear-layer bias is dropped since
    BatchNorm cancels it.
"""
import sys

sys.path.insert(0, "/opt/trn_rl_repo")

import numpy as np

from concourse import bass, bacc, mybir, tile
from concourse.bass_utils import run_bass_kernel_spmd

# problem sizes (hardcoded per spec)
N = 50000
M = 24
A = 64  # atom_fea_len
B = 32  # nbr_fea_len
F = 2 * A + B  # 160
J = 2 * A  # 128
EPS = 1e-5

CORES = 8
P = 128
NS = N // CORES  # 6250
T = (NS + P - 1) // P  # 49 tiles
NPAD = T * P  # 6272

f32 = mybir.dt.float32
i32 = mybir.dt.int32


def build_graph(n=N, m=M, cores=CORES):
    """Build the SPMD Tile graph. Parameterized so a scaled-down version can be
    simulated; the real kernel uses the module constants."""
    ns = n // cores
    t_tiles = (ns + P - 1) // P
    npad = t_tiles * P

    nc_ = bacc.Bacc("TRN2", target_bir_lowering=False, debug=False, num_devices=cores)
    tc = tile.TileContext(nc_)
    tc.__enter__()
    nc = tc.nc

    # ---- DRAM parameters (per-core shards supplied via in_maps) ----
    atab_d = nc.dram_tensor("atab", [n, A], f32, kind="ExternalInput")
    aself_d = nc.dram_tensor("aself", [npad, A], f32, kind="ExternalInput")
    nbr_d = nc.dram_tensor("nbr", [npad, m * B], f32, kind="ExternalInput")
    bwi_d = nc.dram_tensor("bwi", [npad, m], f32, kind="ExternalInput")
    bwj_d = nc.dram_tensor("bwj", [npad, m], f32, kind="ExternalInput")
    idx_d = nc.dram_tensor("idx", [npad, m], i32, kind="ExternalInput")
    validT_d = nc.dram_tensor("validT", [P, t_tiles], f32, kind="ExternalInput")
    wt1_d = nc.dram_tensor("wt1", [J, J], f32, kind="ExternalInput")
    wt2_d = nc.dram_tensor("wt2", [B, J], f32, kind="ExternalInput")
    g1b1_d = nc.dram_tensor("g1b1", [2, J], f32, kind="ExternalInput")
    g2b2_d = nc.dram_tensor("g2b2", [2, A], f32, kind="ExternalInput")
    ident_d = nc.dram_tensor("ident", [P, P], f32, kind="ExternalInput")
    out_d = nc.dram_tensor("out", [npad, A], f32, kind="ExternalOutput")

    rg = [list(range(cores))]

    from contextlib import ExitStack

    es_main = ExitStack()
    const = es_main.enter_context(tc.tile_pool(name="const", bufs=1))
    persist = es_main.enter_context(tc.tile_pool(name="persist", bufs=1))

    # constants
    ident = const.tile([P, P], f32)
    nc.sync.dma_start(out=ident[:], in_=ident_d[:])
    ones1 = const.tile([1, P], f32)
    nc.vector.memset(ones1[:], 1.0)
    wt1_sb = const.tile([J, J], f32)
    nc.sync.dma_start(out=wt1_sb[:], in_=wt1_d[:])
    wt2_sb = const.tile([B, J], f32)
    nc.sync.dma_start(out=wt2_sb[:], in_=wt2_d[:])
    gamma1_sb = const.tile([1, J], f32)
    nc.sync.dma_start(out=gamma1_sb[:], in_=g1b1_d[0:1, :])
    beta1_sb = const.tile([1, J], f32)
    nc.sync.dma_start(out=beta1_sb[:], in_=g1b1_d[1:2, :])
    gamma2_sb = const.tile([1, A], f32)
    nc.sync.dma_start(out=gamma2_sb[:], in_=g2b2_d[0:1, :])
    beta2_sb = const.tile([1, A], f32)
    nc.sync.dma_start(out=beta2_sb[:], in_=g2b2_d[1:2, :])
    validT_sb = const.tile([P, t_tiles], f32)
    nc.sync.dma_start(out=validT_sb[:], in_=validT_d[:])

    # persistent activations
    z_all = persist.tile([P, t_tiles * J], f32)
    core_all = persist.tile([P, t_tiles * A], f32)
    sig_all = persist.tile([P, t_tiles * A], f32)
    ep_all = persist.tile([P, t_tiles * A], f32)
    sq_all = persist.tile([P, t_tiles * A], f32)

    # all gather indices resident up-front so the Pool queue never waits on Sync
    # (tile 0's slice loads first so the gather stream starts immediately)
    idx_all = persist.tile([P, t_tiles * m], i32)
    idx_v = idx_all[:].rearrange("p (t c) -> p t c", c=m)
    idx_dv = idx_d[:].rearrange("(t p) c -> p t c", p=P)
    nc.sync.dma_start(out=idx_v[:, 0:1, :], in_=idx_dv[:, 0:1, :])
    nc.sync.dma_start(out=idx_v[:, 1:t_tiles, :], in_=idx_dv[:, 1:t_tiles, :])

    # ---------------- phase 1: message passing + linear + BN1 stats ----------
    es1 = ExitStack()
    ph1 = es1.enter_context(tc.tile_pool(name="ph1", bufs=2))
    phg = es1.enter_context(tc.tile_pool(name="phg", bufs=3))
    psum1 = es1.enter_context(tc.tile_pool(name="psum1", bufs=2, space="PSUM"))
    psum_acc = es1.enter_context(tc.tile_pool(name="psum_acc", bufs=1, space="PSUM"))

    statz = psum_acc.tile([1, J], f32, name="statz")
    statz2 = psum_acc.tile([1, J], f32, name="statz2")

    for t in range(t_tiles):
        r0 = t * P
        # loads
        bwi_sb = ph1.tile([P, m], f32, name="bwi_sb")
        nc.sync.dma_start(out=bwi_sb[:], in_=bwi_d[r0 : r0 + P, :])
        bwj_sb = ph1.tile([P, m], f32, name="bwj_sb")
        nc.sync.dma_start(out=bwj_sb[:], in_=bwj_d[r0 : r0 + P, :])
        aself_sb = ph1.tile([P, A], f32, name="aself_sb")
        nc.sync.dma_start(out=aself_sb[:], in_=aself_d[r0 : r0 + P, :])
        nbr_sb = ph1.tile([P, m * B], f32, name="nbr_sb")
        nc.sync.dma_start(out=nbr_sb[:], in_=nbr_d[r0 : r0 + P, :])

        # gather: G[p, c, :] = atab[idx[p, c], :]   (HW: one offset per partition
        # per indirect DMA, so one call per neighbor column)
        G = phg.tile([P, m * A], f32, name="G")
        Gv = G[:].rearrange("p (c e) -> p c e", e=A)
        for c in range(m):
            nc.gpsimd.indirect_dma_start(
                out=Gv[:, c, :],
                out_offset=None,
                in_=atab_d[:],
                in_offset=bass.IndirectOffsetOnAxis(
                    ap=idx_all[:, t * m + c : t * m + c + 1], axis=0
                ),
            )

        # w = bwi * bwj ; s = sum_m w
        w_sb = ph1.tile([P, m], f32, name="w_sb")
        nc.vector.tensor_tensor(out=w_sb[:], in0=bwi_sb[:], in1=bwj_sb[:], op=mybir.AluOpType.mult)
        s_sb = ph1.tile([P, 1], f32, name="s_sb")
        nc.vector.reduce_sum(out=s_sb[:], in_=w_sb[:], axis=mybir.AxisListType.X)

        tg = ph1.tile([P, F], f32, name="tg")

        # self part: tg[:, :A] = aself * s   (ACT per-partition scale)
        nc.scalar.mul(tg[:, 0:A], aself_sb[:], s_sb[:, 0:1])

        # neighbor part: prod[p, g, a] = G[p, g, a] * w[p, g] ; sum over g (24)
        prod = ph1.tile([P, m * A], f32, name="prod")
        nc.vector.tensor_tensor(
            out=prod[:],
            in0=G[:],
            in1=w_sb[:].unsqueeze(2).to_broadcast([P, m, A]),
            op=mybir.AluOpType.mult,
        )
        # reduce tree over g: 24 -> 12 -> 6 -> 3 -> 1
        src = prod[:].rearrange("p (g a) -> p g a", a=A)
        g_cnt = m
        lvl_i = 0
        while g_cnt > 3:
            half = g_cnt // 2
            nxt = ph1.tile([P, half * A], f32, name=f"nlvl{lvl_i}", tag=f"nlvl{lvl_i}")
            nc.vector.tensor_tensor(
                out=nxt[:], in0=src[:, 0:half, :], in1=src[:, half : 2 * half, :],
                op=mybir.AluOpType.add,
            )
            src = nxt[:].rearrange("p (g a) -> p g a", a=A)
            g_cnt = half
            lvl_i += 1
        assert g_cnt == 3
        nl = ph1.tile([P, A], f32, name="nl")
        nc.vector.tensor_tensor(out=nl[:], in0=src[:, 0, :], in1=src[:, 1, :], op=mybir.AluOpType.add)
        nc.vector.tensor_tensor(out=tg[:, A : 2 * A], in0=nl[:], in1=src[:, 2, :], op=mybir.AluOpType.add)

        # bond part: bprod[p, mm, b] = nbr[p, mm, b] * w[p, mm]; sum over mm (24)
        bprod = ph1.tile([P, m * B], f32, name="bprod")
        nc.vector.tensor_tensor(
            out=bprod[:],
            in0=nbr_sb[:],
            in1=w_sb[:].unsqueeze(2).to_broadcast([P, m, B]),
            op=mybir.AluOpType.mult,
        )
        bsrc = bprod[:].rearrange("p (g b) -> p g b", b=B)
        g_cnt = m
        lvl_i = 0
        while g_cnt > 3:
            half = g_cnt // 2
            nxt = ph1.tile([P, half * B], f32, name=f"blvl{lvl_i}", tag=f"blvl{lvl_i}")
            nc.vector.tensor_tensor(
                out=nxt[:], in0=bsrc[:, 0:half, :], in1=bsrc[:, half : 2 * half, :],
                op=mybir.AluOpType.add,
            )
            bsrc = nxt[:].rearrange("p (g b) -> p g b", b=B)
            g_cnt = half
            lvl_i += 1
        assert g_cnt == 3
        bl = ph1.tile([P, B], f32, name="bl")
        nc.vector.tensor_tensor(out=bl[:], in0=bsrc[:, 0, :], in1=bsrc[:, 1, :], op=mybir.AluOpType.add)
        nc.vector.tensor_tensor(out=tg[:, 2 * A : F], in0=bl[:], in1=bsrc[:, 2, :], op=mybir.AluOpType.add)

        # transpose tg -> tgT (two chunks), then z = tg @ W^T
        pT1 = psum1.tile([P, P], f32, name="pT1")
        nc.tensor.transpose(out=pT1[:], in_=tg[:, 0:J], identity=ident[:])
        pT2 = psum1.tile([B, P], f32, name="pT2")
        nc.tensor.transpose(out=pT2[:], in_=tg[:, J:F], identity=ident[:])
        tgT1 = ph1.tile([P, P], f32, name="tgT1")
        nc.vector.tensor_copy(out=tgT1[:], in_=pT1[:])
        tgT2 = ph1.tile([B, P], f32, name="tgT2")
        nc.vector.tensor_copy(out=tgT2[:], in_=pT2[:])

        zp = psum1.tile([P, J], f32, name="zp")
        nc.tensor.matmul(out=zp[:], lhsT=tgT1[:], rhs=wt1_sb[:], start=True, stop=False)
        nc.tensor.matmul(out=zp[:], lhsT=tgT2[:], rhs=wt2_sb[:], start=False, stop=True)

        z_sl = z_all[:, t * J : (t + 1) * J]
        nc.vector.tensor_copy(out=z_sl, in_=zp[:])
        z2_sb = ph1.tile([P, J], f32, name="z2_sb")
        nc.scalar.square(z2_sb[:], zp[:])

        # BN1 partial stats (masked partition sums, accumulated in PSUM)
        vcol = validT_sb[:, t : t + 1]
        nc.tensor.matmul(
            out=statz[:], lhsT=vcol, rhs=z_sl, start=(t == 0), stop=(t == t_tiles - 1),
            skip_group_check=True,
        )
        nc.tensor.matmul(
            out=statz2[:], lhsT=vcol, rhs=z2_sb[:], start=(t == 0), stop=(t == t_tiles - 1),
            skip_group_check=True,
        )

    # ---- AllReduce BN1 stats ----
    s1_pack = persist.tile([1, 2 * J], f32)
    nc.vector.tensor_copy(out=s1_pack[:, 0:J], in_=statz[:])
    nc.vector.tensor_copy(out=s1_pack[:, J : 2 * J], in_=statz2[:])

    dram = es_main.enter_context(tc.tile_pool(name="dram", bufs=1, space="DRAM"))
    ar1_in = dram.tile([1, 2 * J], f32)
    ar1_out = dram.tile([1, 2 * J], f32, addr_space="Shared")
    nc.sync.dma_start(out=ar1_in[:], in_=s1_pack[:])
    nc.gpsimd.collective_compute(
        "AllReduce", mybir.AluOpType.add, replica_groups=rg,
        ins=[ar1_in[:].opt()], outs=[ar1_out[:].opt()],
    )
    s1g = persist.tile([1, 2 * J], f32)
    nc.sync.dma_start(out=s1g[:], in_=ar1_out[:])
    sum1g = s1g[:, 0:J]
    sq1g = s1g[:, J : 2 * J]

    # ---- BN1 affine coefficients + broadcast ----
    es1.close()

    coef = es_main.enter_context(tc.tile_pool(name="coef", bufs=1))
    psum_b = es_main.enter_context(tc.tile_pool(name="psum_b", bufs=1, space="PSUM"))

    inv_n = 1.0 / float(n)
    mean1 = coef.tile([1, J], f32)
    nc.scalar.mul(mean1[:], sum1g, inv_n)
    ex2 = coef.tile([1, J], f32)
    nc.scalar.mul(ex2[:], sq1g, inv_n)
    msq = coef.tile([1, J], f32)
    nc.vector.tensor_tensor(out=msq[:], in0=mean1[:], in1=mean1[:], op=mybir.AluOpType.mult)
    var1 = coef.tile([1, J], f32)
    nc.vector.tensor_tensor(out=var1[:], in0=ex2[:], in1=msq[:], op=mybir.AluOpType.subtract)
    nc.vector.tensor_scalar_add(var1[:], var1[:], EPS)
    lnv1 = coef.tile([1, J], f32)
    nc.scalar.activation(lnv1[:], var1[:], mybir.ActivationFunctionType.Ln)
    rstd1 = coef.tile([1, J], f32)
    nc.scalar.activation(rstd1[:], lnv1[:], mybir.ActivationFunctionType.Exp, scale=-0.5)
    # a1 = gamma1 * rstd ; c1 = beta1 - mean * a1  (packed [1, 2J])
    a1c1 = coef.tile([1, 2 * J], f32)
    nc.vector.tensor_tensor(out=a1c1[:, 0:J], in0=gamma1_sb[:], in1=rstd1[:], op=mybir.AluOpType.mult)
    ma1 = coef.tile([1, J], f32)
    nc.vector.tensor_tensor(out=ma1[:], in0=mean1[:], in1=a1c1[:, 0:J], op=mybir.AluOpType.mult)
    nc.vector.tensor_tensor(out=a1c1[:, J : 2 * J], in0=beta1_sb[:], in1=ma1[:], op=mybir.AluOpType.subtract)

    bc1p = psum_b.tile([P, 2 * J], f32)
    nc.tensor.matmul(out=bc1p[:], lhsT=ones1[:], rhs=a1c1[:], start=True, stop=True)
    A1C1 = persist.tile([P, 2 * J], f32)
    nc.vector.tensor_copy(out=A1C1[:], in_=bc1p[:])

    # ---------------- phase 2: BN1 apply + gating + BN2 stats ----------------
    es2 = ExitStack()
    ph2 = es2.enter_context(tc.tile_pool(name="ph2", bufs=2))
    psum2 = es2.enter_context(tc.tile_pool(name="psum2", bufs=1, space="PSUM"))
    statc = psum2.tile([1, A], f32, name="statc")
    statc2 = psum2.tile([1, A], f32, name="statc2")

    # phase 2 in two half-core chunks so ACT work overlaps DVE work
    a1_full = A1C1[:, 0:J]
    c1_full = A1C1[:, J : 2 * J]
    half = t_tiles // 2
    for lo, hi in ((0, half), (half, t_tiles)):
        nt = hi - lo
        znv = z_all[:, lo * J : hi * J].rearrange("p (t j) -> p t j", j=J)
        a1_per = a1_full.unsqueeze(1).to_broadcast([P, nt, J])
        c1_per = c1_full.unsqueeze(1).to_broadcast([P, nt, J])
        nc.vector.tensor_tensor(out=znv, in0=znv, in1=a1_per, op=mybir.AluOpType.mult)
        nc.vector.tensor_tensor(out=znv, in0=znv, in1=c1_per, op=mybir.AluOpType.add)
        nc.scalar.activation(
            sig_all[:, lo * A : hi * A].rearrange("p (t a) -> p t a", a=A),
            znv[:, :, 0:A],
            mybir.ActivationFunctionType.Sigmoid,
        )
        nc.scalar.activation(
            ep_all[:, lo * A : hi * A].rearrange("p (t a) -> p t a", a=A),
            znv[:, :, A:J],
            mybir.ActivationFunctionType.Exp,
        )
        nc.scalar.activation(
            ep_all[:, lo * A : hi * A],
            ep_all[:, lo * A : hi * A],
            mybir.ActivationFunctionType.Ln,
            bias=1.0,
        )
        nc.vector.tensor_tensor(
            out=core_all[:, lo * A : hi * A],
            in0=sig_all[:, lo * A : hi * A],
            in1=ep_all[:, lo * A : hi * A],
            op=mybir.AluOpType.mult,
        )
        vmask = validT_sb[:, lo:hi].unsqueeze(2).to_broadcast([P, nt, A])
        nc.vector.tensor_tensor(
            out=core_all[:, lo * A : hi * A].rearrange("p (t a) -> p t a", a=A),
            in0=core_all[:, lo * A : hi * A].rearrange("p (t a) -> p t a", a=A),
            in1=vmask,
            op=mybir.AluOpType.mult,
        )
        nc.scalar.square(sq_all[:, lo * A : hi * A], core_all[:, lo * A : hi * A])
    # per-partition sums over t, then partition-sum via ones-matmul
    ms_c = persist.tile([P, A], f32)
    nc.vector.reduce_sum(
        out=ms_c[:],
        in_=core_all[:].rearrange("p (t a) -> p a t", a=A),
        axis=mybir.AxisListType.X,
    )
    ms_c2 = persist.tile([P, A], f32)
    nc.vector.reduce_sum(
        out=ms_c2[:],
        in_=sq_all[:].rearrange("p (t a) -> p a t", a=A),
        axis=mybir.AxisListType.X,
    )
    ones128 = const.tile([P, 1], f32)
    nc.vector.memset(ones128[:], 1.0)
    statc = psum2.tile([1, A], f32, name="statc")
    statc2 = psum2.tile([1, A], f32, name="statc2")
    nc.tensor.matmul(out=statc[:], lhsT=ones128[:], rhs=ms_c[:], start=True, stop=True)
    nc.tensor.matmul(out=statc2[:], lhsT=ones128[:], rhs=ms_c2[:], start=True, stop=True)

    # ---- AllReduce BN2 stats ----
    s2_pack = persist.tile([1, 2 * A], f32)
    nc.vector.tensor_copy(out=s2_pack[:, 0:A], in_=statc[:])
    nc.vector.tensor_copy(out=s2_pack[:, A : 2 * A], in_=statc2[:])
    ar2_in = dram.tile([1, 2 * A], f32)
    ar2_out = dram.tile([1, 2 * A], f32, addr_space="Shared")
    nc.sync.dma_start(out=ar2_in[:], in_=s2_pack[:])
    nc.gpsimd.collective_compute(
        "AllReduce", mybir.AluOpType.add, replica_groups=rg,
        ins=[ar2_in[:].opt()], outs=[ar2_out[:].opt()],
    )
    s2g = persist.tile([1, 2 * A], f32)
    nc.sync.dma_start(out=s2g[:], in_=ar2_out[:])
    sum2g_ap = s2g[:, 0:A]
    sq2g_ap = s2g[:, A : 2 * A]

    mean2 = coef.tile([1, A], f32)
    nc.scalar.mul(mean2[:], sum2g_ap, inv_n)
    ex22 = coef.tile([1, A], f32)
    nc.scalar.mul(ex22[:], sq2g_ap, inv_n)
    msq2 = coef.tile([1, A], f32)
    nc.vector.tensor_tensor(out=msq2[:], in0=mean2[:], in1=mean2[:], op=mybir.AluOpType.mult)
    var2 = coef.tile([1, A], f32)
    nc.vector.tensor_tensor(out=var2[:], in0=ex22[:], in1=msq2[:], op=mybir.AluOpType.subtract)
    nc.vector.tensor_scalar_add(var2[:], var2[:], EPS)
    lnv2 = coef.tile([1, A], f32)
    nc.scalar.activation(lnv2[:], var2[:], mybir.ActivationFunctionType.Ln)
    rstd2 = coef.tile([1, A], f32)
    nc.scalar.activation(rstd2[:], lnv2[:], mybir.ActivationFunctionType.Exp, scale=-0.5)
    a2c2 = coef.tile([1, 2 * A], f32)
    nc.vector.tensor_tensor(out=a2c2[:, 0:A], in0=gamma2_sb[:], in1=rstd2[:], op=mybir.AluOpType.mult)
    ma2 = coef.tile([1, A], f32)
    nc.vector.tensor_tensor(out=ma2[:], in0=mean2[:], in1=a2c2[:, 0:A], op=mybir.AluOpType.mult)
    nc.vector.tensor_tensor(out=a2c2[:, A : 2 * A], in0=beta2_sb[:], in1=ma2[:], op=mybir.AluOpType.subtract)

    bc2p = psum_b.tile([P, 2 * A], f32)
    nc.tensor.matmul(out=bc2p[:], lhsT=ones1[:], rhs=a2c2[:], start=True, stop=True)
    A2C2 = persist.tile([P, 2 * A], f32)
    nc.vector.tensor_copy(out=A2C2[:], in_=bc2p[:])

    # ---------------- phase 3: BN2 apply + softplus + store ----------------
    a2_full = A2C2[:, 0:A]
    c2_full = A2C2[:, A : 2 * A]
    for lo, hi in ((0, half), (half, t_tiles)):
        nt = hi - lo
        corev = core_all[:, lo * A : hi * A].rearrange("p (t a) -> p t a", a=A)
        a2_per = a2_full.unsqueeze(1).to_broadcast([P, nt, A])
        c2_per = c2_full.unsqueeze(1).to_broadcast([P, nt, A])
        nc.vector.tensor_tensor(out=corev, in0=corev, in1=a2_per, op=mybir.AluOpType.mult)
        nc.vector.tensor_tensor(out=corev, in0=corev, in1=c2_per, op=mybir.AluOpType.add)
        nc.scalar.activation(
            core_all[:, lo * A : hi * A],
            core_all[:, lo * A : hi * A],
            mybir.ActivationFunctionType.Exp,
        )
        nc.scalar.activation(
            core_all[:, lo * A : hi * A],
            core_all[:, lo * A : hi * A],
            mybir.ActivationFunctionType.Ln,
            bias=1.0,
        )
        nc.sync.dma_start(
            out=out_d[lo * P * 1 :, :].rearrange("(t p) a -> p t a", p=P)[:, 0:nt, :]
            if lo
            else out_d[0 : hi * P, :].rearrange("(t p) a -> p t a", p=P),
            in_=corev,
        )

    es2.close()
    es_main.close()
    tc.__exit__(None, None, None)
    nc_.compile()
    return nc_


def make_in_maps(inputs, n=N, m=M, cores=CORES):
    """Host-side sharding/layout prep (index reshuffling + dtype conversion only)."""
    ns = n // cores
    t_tiles = (ns + P - 1) // P
    npad = t_tiles * P

    atom = np.asarray(inputs["atom_in_fea"], np.float32)
    nbr = np.asarray(inputs["nbr_fea"], np.float32).reshape(n, m * B)
    idx = np.asarray(inputs["nbr_fea_idx"])
    bwi = np.asarray(inputs["bond_weights_i"], np.float32)
    bwj = np.asarray(inputs["bond_weights_j"], np.float32)
    W = np.asarray(inputs["W"], np.float32)
    g1 = np.asarray(inputs["gamma1"], np.float32)
    b1 = np.asarray(inputs["beta1"], np.float32)
    g2 = np.asarray(inputs["gamma2"], np.float32)
    b2 = np.asarray(inputs["beta2"], np.float32)

    Wt = np.ascontiguousarray(W.T)  # [F, J]
    wt1 = np.ascontiguousarray(Wt[0:J, :])
    wt2 = np.ascontiguousarray(Wt[J:F, :])
    g1b1 = np.stack([g1, b1]).astype(np.float32)
    g2b2 = np.stack([g2, b2]).astype(np.float32)

    valid = np.zeros((npad,), np.float32)
    valid[:ns] = 1.0
    validT = np.ascontiguousarray(valid.reshape(t_tiles, P).T)

    in_maps = []
    for c in range(cores):
        lo, hi = c * ns, (c + 1) * ns
        pad = npad - ns

        def padrows(x):
            return np.concatenate([x, np.zeros((pad,) + x.shape[1:], x.dtype)], 0) if pad else x

        idx_c = np.concatenate([idx[lo:hi], np.zeros((pad, m), idx.dtype)], 0) if pad else idx[lo:hi]

        in_maps.append(
            {
                "atab": atom,
                "aself": padrows(atom[lo:hi]),
                "nbr": padrows(nbr[lo:hi]),
                "bwi": padrows(bwi[lo:hi]),
                "bwj": padrows(bwj[lo:hi]),
                "idx": np.ascontiguousarray(idx_c.astype(np.int32)),
                "validT": validT,
                "wt1": wt1,
                "wt2": wt2,
                "g1b1": g1b1,
                "g2b2": g2b2,
                "ident": np.eye(P, dtype=np.float32),
            }
        )
    return in_maps


_GRAPH_CACHE = {}


def _get_graph():
    if "nc" not in _GRAPH_CACHE:
        _GRAPH_CACHE["nc"] = build_graph()
    return _GRAPH_CACHE["nc"]


def run(inputs, trace=False, **kw):
    nc = _get_graph()
    in_maps = make_in_maps(inputs)
    res = run_bass_kernel_spmd(nc, in_maps, core_ids=list(range(CORES)), trace=trace, **kw)
    ns = N // CORES
    out = np.concatenate([res.results[c]["out"][:ns] for c in range(CORES)], 0)
    return out.astype(np.float32), res


def kernel(**inputs) -> np.ndarray:
    out, _ = run(inputs, trace=False)
    return out


# revision 17
# speedup vs baseline: 1.3672x; 1.0040x over previous
"""AtomConvLayer (CGCNN message passing) distributed Bass kernel for 8 TRN2 NeuronCores.

Strategy (data-parallel over atoms N):
  - Each core owns N/8 = 6250 atom rows (padded to 6272 = 49*128); the atom
    feature table is replicated to every core for the neighbor gather.
  - The gather atom_in_fea[nbr_fea_idx] runs on-device via indirect DMA
    (`indirect_dma_start`, one offset per partition -> 128 rows/call; 24 calls
    per 128-row tile). This image lacks the custom GPSIMD ucode (dma_gather),
    so the Q7 SWDGE descriptor loop is the kernel's critical path.
  - Per 128-row tile: w = bwi*bwj; self/nbr/bond parts reduced on DVE with
    broadcast-AP multiplies and contiguous add-trees; concat -> PE transpose ->
    matmul with W^T -> z kept resident in SBUF.
  - BatchNorm stats are partition-reduced per tile with a valid-mask matmul
    accumulated in PSUM, then AllReduce'd across the 8 cores (sum|sumsq packed).
  - BN1 affine + sigmoid*softplus and BN2 + softplus run as whole-core mega-ops
    (single instructions over all 49 tiles with periodic broadcast APs) to keep
    the post-collective tail short; the linear-layer bias is dropped since
    BatchNorm cancels it.
"""
import sys

sys.path.insert(0, "/opt/trn_rl_repo")

import numpy as np
import ml_dtypes

from concourse import bass, bacc, mybir, tile
from concourse.bass_utils import run_bass_kernel_spmd

# problem sizes (hardcoded per spec)
N = 50000
M = 24
A = 64  # atom_fea_len
B = 32  # nbr_fea_len
F = 2 * A + B  # 160
J = 2 * A  # 128
EPS = 1e-5

CORES = 8
P = 128
NS = N // CORES  # 6250
T = (NS + P - 1) // P  # 49 tiles
NPAD = T * P  # 6272

f32 = mybir.dt.float32
bf16 = mybir.dt.bfloat16
i32 = mybir.dt.int32


def build_graph(n=N, m=M, cores=CORES):
    """Build the SPMD Tile graph. Parameterized so a scaled-down version can be
    simulated; the real kernel uses the module constants."""
    ns = n // cores
    t_tiles = (ns + P - 1) // P
    npad = t_tiles * P

    nc_ = bacc.Bacc("TRN2", target_bir_lowering=False, debug=False, num_devices=cores)
    tc = tile.TileContext(nc_)
    tc.__enter__()
    nc = tc.nc

    # ---- DRAM parameters (per-core shards supplied via in_maps) ----
    atab_d = nc.dram_tensor("atab", [n, A], bf16, kind="ExternalInput")
    aself_d = nc.dram_tensor("aself", [npad, A], f32, kind="ExternalInput")
    nbr_d = nc.dram_tensor("nbr", [npad, m * B], f32, kind="ExternalInput")
    bwi_d = nc.dram_tensor("bwi", [npad, m], f32, kind="ExternalInput")
    bwj_d = nc.dram_tensor("bwj", [npad, m], f32, kind="ExternalInput")
    idx_d = nc.dram_tensor("idx", [npad, m], i32, kind="ExternalInput")
    validT_d = nc.dram_tensor("validT", [P, t_tiles], f32, kind="ExternalInput")
    wt1_d = nc.dram_tensor("wt1", [J, J], f32, kind="ExternalInput")
    wt2_d = nc.dram_tensor("wt2", [B, J], f32, kind="ExternalInput")
    g1b1_d = nc.dram_tensor("g1b1", [2, J], f32, kind="ExternalInput")
    g2b2_d = nc.dram_tensor("g2b2", [2, A], f32, kind="ExternalInput")
    ident_d = nc.dram_tensor("ident", [P, P], f32, kind="ExternalInput")
    out_d = nc.dram_tensor("out", [npad, A], f32, kind="ExternalOutput")

    rg = [list(range(cores))]

    from contextlib import ExitStack

    es_main = ExitStack()
    const = es_main.enter_context(tc.tile_pool(name="const", bufs=1))
    persist = es_main.enter_context(tc.tile_pool(name="persist", bufs=1))

    # constants
    ident = const.tile([P, P], f32)
    nc.sync.dma_start(out=ident[:], in_=ident_d[:])
    ones1 = const.tile([1, P], f32)
    nc.vector.memset(ones1[:], 1.0)
    wt1_sb = const.tile([J, J], f32)
    nc.sync.dma_start(out=wt1_sb[:], in_=wt1_d[:])
    wt2_sb = const.tile([B, J], f32)
    nc.sync.dma_start(out=wt2_sb[:], in_=wt2_d[:])
    gamma1_sb = const.tile([1, J], f32)
    nc.sync.dma_start(out=gamma1_sb[:], in_=g1b1_d[0:1, :])
    beta1_sb = const.tile([1, J], f32)
    nc.sync.dma_start(out=beta1_sb[:], in_=g1b1_d[1:2, :])
    gamma2_sb = const.tile([1, A], f32)
    nc.sync.dma_start(out=gamma2_sb[:], in_=g2b2_d[0:1, :])
    beta2_sb = const.tile([1, A], f32)
    nc.sync.dma_start(out=beta2_sb[:], in_=g2b2_d[1:2, :])
    validT_sb = const.tile([P, t_tiles], f32)
    nc.sync.dma_start(out=validT_sb[:], in_=validT_d[:])

    # persistent activations
    z_all = persist.tile([P, t_tiles * J], f32)
    core_all = persist.tile([P, t_tiles * A], f32)
    sig_all = persist.tile([P, t_tiles * A], f32)
    ep_all = persist.tile([P, t_tiles * A], f32)
    sq_all = persist.tile([P, t_tiles * A], f32)

    # all gather indices resident up-front so the Pool queue never waits on Sync
    # (tile 0's slice loads first so the gather stream starts immediately)
    idx_all = persist.tile([P, t_tiles * m], i32)
    idx_v = idx_all[:].rearrange("p (t c) -> p t c", c=m)
    idx_dv = idx_d[:].rearrange("(t p) c -> p t c", p=P)
    nc.sync.dma_start(out=idx_v[:, 0:1, :], in_=idx_dv[:, 0:1, :])
    nc.sync.dma_start(out=idx_v[:, 1:t_tiles, :], in_=idx_dv[:, 1:t_tiles, :])

    # ---------------- phase 1: message passing + linear + BN1 stats ----------
    es1 = ExitStack()
    ph1 = es1.enter_context(tc.tile_pool(name="ph1", bufs=2))
    phg = es1.enter_context(tc.tile_pool(name="phg", bufs=3))
    psum1 = es1.enter_context(tc.tile_pool(name="psum1", bufs=2, space="PSUM"))
    psum_acc = es1.enter_context(tc.tile_pool(name="psum_acc", bufs=1, space="PSUM"))

    statz = psum_acc.tile([1, J], f32, name="statz")
    statz2 = psum_acc.tile([1, J], f32, name="statz2")

    for t in range(t_tiles):
        r0 = t * P
        # loads
        bwi_sb = ph1.tile([P, m], f32, name="bwi_sb")
        nc.sync.dma_start(out=bwi_sb[:], in_=bwi_d[r0 : r0 + P, :])
        bwj_sb = ph1.tile([P, m], f32, name="bwj_sb")
        nc.sync.dma_start(out=bwj_sb[:], in_=bwj_d[r0 : r0 + P, :])
        aself_sb = ph1.tile([P, A], f32, name="aself_sb")
        nc.sync.dma_start(out=aself_sb[:], in_=aself_d[r0 : r0 + P, :])
        nbr_sb = ph1.tile([P, m * B], f32, name="nbr_sb")
        nc.sync.dma_start(out=nbr_sb[:], in_=nbr_d[r0 : r0 + P, :])

        # gather: G[p, c, :] = atab[idx[p, c], :]   (HW: one offset per partition
        # per indirect DMA, so one call per neighbor column)
        G = phg.tile([P, m * A], bf16, name="G")
        Gv = G[:].rearrange("p (c e) -> p c e", e=A)
        for c in range(m):
            nc.gpsimd.indirect_dma_start(
                out=Gv[:, c, :],
                out_offset=None,
                in_=atab_d[:],
                in_offset=bass.IndirectOffsetOnAxis(
                    ap=idx_all[:, t * m + c : t * m + c + 1], axis=0
                ),
            )

        # w = bwi * bwj ; s = sum_m w
        w_sb = ph1.tile([P, m], f32, name="w_sb")
        nc.vector.tensor_tensor(out=w_sb[:], in0=bwi_sb[:], in1=bwj_sb[:], op=mybir.AluOpType.mult)
        s_sb = ph1.tile([P, 1], f32, name="s_sb")
        nc.vector.reduce_sum(out=s_sb[:], in_=w_sb[:], axis=mybir.AxisListType.X)

        tg = ph1.tile([P, F], f32, name="tg")

        # self part: tg[:, :A] = aself * s   (ACT per-partition scale)
        nc.scalar.mul(tg[:, 0:A], aself_sb[:], s_sb[:, 0:1])

        # neighbor part: prod[p, g, a] = G[p, g, a] * w[p, g] ; sum over g (24)
        prod = ph1.tile([P, m * A], f32, name="prod")
        nc.vector.tensor_tensor(
            out=prod[:],
            in0=G[:],
            in1=w_sb[:].unsqueeze(2).to_broadcast([P, m, A]),
            op=mybir.AluOpType.mult,
        )
        # reduce tree over g: 24 -> 12 -> 6 -> 3 -> 1
        src = prod[:].rearrange("p (g a) -> p g a", a=A)
        g_cnt = m
        lvl_i = 0
        while g_cnt > 3:
            half = g_cnt // 2
            nxt = ph1.tile([P, half * A], f32, name=f"nlvl{lvl_i}", tag=f"nlvl{lvl_i}")
            nc.vector.tensor_tensor(
                out=nxt[:], in0=src[:, 0:half, :], in1=src[:, half : 2 * half, :],
                op=mybir.AluOpType.add,
            )
            src = nxt[:].rearrange("p (g a) -> p g a", a=A)
            g_cnt = half
            lvl_i += 1
        assert g_cnt == 3
        nl = ph1.tile([P, A], f32, name="nl")
        nc.vector.tensor_tensor(out=nl[:], in0=src[:, 0, :], in1=src[:, 1, :], op=mybir.AluOpType.add)
        nc.vector.tensor_tensor(out=tg[:, A : 2 * A], in0=nl[:], in1=src[:, 2, :], op=mybir.AluOpType.add)

        # bond part: bprod[p, mm, b] = nbr[p, mm, b] * w[p, mm]; sum over mm (24)
        bprod = ph1.tile([P, m * B], f32, name="bprod")
        nc.vector.tensor_tensor(
            out=bprod[:],
            in0=nbr_sb[:],
            in1=w_sb[:].unsqueeze(2).to_broadcast([P, m, B]),
            op=mybir.AluOpType.mult,
        )
        bsrc = bprod[:].rearrange("p (g b) -> p g b", b=B)
        g_cnt = m
        lvl_i = 0
        while g_cnt > 3:
            half = g_cnt // 2
            nxt = ph1.tile([P, half * B], f32, name=f"blvl{lvl_i}", tag=f"blvl{lvl_i}")
            nc.vector.tensor_tensor(
                out=nxt[:], in0=bsrc[:, 0:half, :], in1=bsrc[:, half : 2 * half, :],
                op=mybir.AluOpType.add,
            )
            bsrc = nxt[:].rearrange("p (g b) -> p g b", b=B)
            g_cnt = half
            lvl_i += 1
        assert g_cnt == 3
        bl = ph1.tile([P, B], f32, name="bl")
        nc.vector.tensor_tensor(out=bl[:], in0=bsrc[:, 0, :], in1=bsrc[:, 1, :], op=mybir.AluOpType.add)
        nc.vector.tensor_tensor(out=tg[:, 2 * A : F], in0=bl[:], in1=bsrc[:, 2, :], op=mybir.AluOpType.add)

        # transpose tg -> tgT (two chunks), then z = tg @ W^T
        pT1 = psum1.tile([P, P], f32, name="pT1")
        nc.tensor.transpose(out=pT1[:], in_=tg[:, 0:J], identity=ident[:])
        pT2 = psum1.tile([B, P], f32, name="pT2")
        nc.tensor.transpose(out=pT2[:], in_=tg[:, J:F], identity=ident[:])
        tgT1 = ph1.tile([P, P], f32, name="tgT1")
        nc.vector.tensor_copy(out=tgT1[:], in_=pT1[:])
        tgT2 = ph1.tile([B, P], f32, name="tgT2")
        nc.vector.tensor_copy(out=tgT2[:], in_=pT2[:])

        zp = psum1.tile([P, J], f32, name="zp")
        nc.tensor.matmul(out=zp[:], lhsT=tgT1[:], rhs=wt1_sb[:], start=True, stop=False)
        nc.tensor.matmul(out=zp[:], lhsT=tgT2[:], rhs=wt2_sb[:], start=False, stop=True)

        z_sl = z_all[:, t * J : (t + 1) * J]
        nc.vector.tensor_copy(out=z_sl, in_=zp[:])
        z2_sb = ph1.tile([P, J], f32, name="z2_sb")
        nc.scalar.square(z2_sb[:], zp[:])

        # BN1 partial stats (masked partition sums, accumulated in PSUM)
        vcol = validT_sb[:, t : t + 1]
        nc.tensor.matmul(
            out=statz[:], lhsT=vcol, rhs=z_sl, start=(t == 0), stop=(t == t_tiles - 1),
            skip_group_check=True,
        )
        nc.tensor.matmul(
            out=statz2[:], lhsT=vcol, rhs=z2_sb[:], start=(t == 0), stop=(t == t_tiles - 1),
            skip_group_check=True,
        )

    # ---- AllReduce BN1 stats ----
    s1_pack = persist.tile([1, 2 * J], f32)
    nc.scalar.copy(s1_pack[:, 0:J], statz[:])
    nc.scalar.copy(s1_pack[:, J : 2 * J], statz2[:])

    dram = es_main.enter_context(tc.tile_pool(name="dram", bufs=1, space="DRAM"))
    ar1_in = dram.tile([1, 2 * J], f32)
    ar1_out = dram.tile([1, 2 * J], f32, addr_space="Shared")
    nc.sync.dma_start(out=ar1_in[:], in_=s1_pack[:])
    nc.gpsimd.collective_compute(
        "AllReduce", mybir.AluOpType.add, replica_groups=rg,
        ins=[ar1_in[:].opt()], outs=[ar1_out[:].opt()],
    )
    s1g = persist.tile([1, 2 * J], f32)
    nc.sync.dma_start(out=s1g[:], in_=ar1_out[:])
    sum1g = s1g[:, 0:J]
    sq1g = s1g[:, J : 2 * J]

    # ---- BN1 affine coefficients + broadcast ----
    es1.close()

    coef = es_main.enter_context(tc.tile_pool(name="coef", bufs=1))
    psum_b = es_main.enter_context(tc.tile_pool(name="psum_b", bufs=1, space="PSUM"))

    inv_n = 1.0 / float(n)
    mean1 = coef.tile([1, J], f32)
    nc.scalar.mul(mean1[:], sum1g, inv_n)
    ex2 = coef.tile([1, J], f32)
    nc.scalar.mul(ex2[:], sq1g, inv_n)
    msq = coef.tile([1, J], f32)
    nc.vector.tensor_tensor(out=msq[:], in0=mean1[:], in1=mean1[:], op=mybir.AluOpType.mult)
    var1 = coef.tile([1, J], f32)
    nc.vector.tensor_tensor(out=var1[:], in0=ex2[:], in1=msq[:], op=mybir.AluOpType.subtract)
    nc.vector.tensor_scalar_add(var1[:], var1[:], EPS)
    lnv1 = coef.tile([1, J], f32)
    nc.scalar.activation(lnv1[:], var1[:], mybir.ActivationFunctionType.Ln)
    rstd1 = coef.tile([1, J], f32)
    nc.scalar.activation(rstd1[:], lnv1[:], mybir.ActivationFunctionType.Exp, scale=-0.5)
    # a1 = gamma1 * rstd ; c1 = beta1 - mean * a1  (packed [1, 2J])
    a1c1 = coef.tile([1, 2 * J], f32)
    nc.vector.tensor_tensor(out=a1c1[:, 0:J], in0=gamma1_sb[:], in1=rstd1[:], op=mybir.AluOpType.mult)
    ma1 = coef.tile([1, J], f32)
    nc.vector.tensor_tensor(out=ma1[:], in0=mean1[:], in1=a1c1[:, 0:J], op=mybir.AluOpType.mult)
    nc.vector.tensor_tensor(out=a1c1[:, J : 2 * J], in0=beta1_sb[:], in1=ma1[:], op=mybir.AluOpType.subtract)

    bc1p = psum_b.tile([P, 2 * J], f32)
    nc.tensor.matmul(out=bc1p[:], lhsT=ones1[:], rhs=a1c1[:], start=True, stop=True)
    A1C1 = persist.tile([P, 2 * J], f32)
    nc.vector.tensor_copy(out=A1C1[:], in_=bc1p[:])

    # ---------------- phase 2: BN1 apply + gating + BN2 stats ----------------
    es2 = ExitStack()
    ph2 = es2.enter_context(tc.tile_pool(name="ph2", bufs=2))
    psum2 = es2.enter_context(tc.tile_pool(name="psum2", bufs=1, space="PSUM"))
    statc = psum2.tile([1, A], f32, name="statc")
    statc2 = psum2.tile([1, A], f32, name="statc2")

    # phase 2: two half-core chunks, ACT ops grouped by function so the
    # act-table pass emits one load per function instead of per chunk
    a1_full = A1C1[:, 0:J]
    c1_full = A1C1[:, J : 2 * J]
    half = t_tiles // 2
    chunks = ((0, half), (half, t_tiles))

    def znv_of(lo, hi):
        return z_all[:, lo * J : hi * J].rearrange("p (t j) -> p t j", j=J)

    for lo, hi in chunks:
        nt = hi - lo
        znv = znv_of(lo, hi)
        a1_per = a1_full.unsqueeze(1).to_broadcast([P, nt, J])
        c1_per = c1_full.unsqueeze(1).to_broadcast([P, nt, J])
        nc.vector.tensor_tensor(out=znv, in0=znv, in1=a1_per, op=mybir.AluOpType.mult)
        nc.vector.tensor_tensor(out=znv, in0=znv, in1=c1_per, op=mybir.AluOpType.add)
    for lo, hi in chunks:
        nc.scalar.activation(
            sig_all[:, lo * A : hi * A].rearrange("p (t a) -> p t a", a=A),
            znv_of(lo, hi)[:, :, 0:A],
            mybir.ActivationFunctionType.Sigmoid,
        )
    for lo, hi in chunks:
        nc.scalar.activation(
            ep_all[:, lo * A : hi * A].rearrange("p (t a) -> p t a", a=A),
            znv_of(lo, hi)[:, :, A:J],
            mybir.ActivationFunctionType.Exp,
        )
    for lo, hi in chunks:
        nc.scalar.activation(
            ep_all[:, lo * A : hi * A],
            ep_all[:, lo * A : hi * A],
            mybir.ActivationFunctionType.Ln,
            bias=1.0,
        )
    for lo, hi in chunks:
        nt = hi - lo
        nc.vector.tensor_tensor(
            out=core_all[:, lo * A : hi * A],
            in0=sig_all[:, lo * A : hi * A],
            in1=ep_all[:, lo * A : hi * A],
            op=mybir.AluOpType.mult,
        )
        vmask = validT_sb[:, lo:hi].unsqueeze(2).to_broadcast([P, nt, A])
        nc.vector.tensor_tensor(
            out=core_all[:, lo * A : hi * A].rearrange("p (t a) -> p t a", a=A),
            in0=core_all[:, lo * A : hi * A].rearrange("p (t a) -> p t a", a=A),
            in1=vmask,
            op=mybir.AluOpType.mult,
        )
    for lo, hi in chunks:
        nc.scalar.square(sq_all[:, lo * A : hi * A], core_all[:, lo * A : hi * A])
    # chunked t-reduction (overlaps with the other chunk's ACT work), then combine
    ms_c = persist.tile([P, A], f32)
    ms_c2 = persist.tile([P, A], f32)
    msp_c = persist.tile([P, A], f32)
    msp_c2 = persist.tile([P, A], f32)
    for i, (lo, hi) in enumerate(chunks):
        dst = ms_c if i == 0 else msp_c
        nc.vector.reduce_sum(
            out=dst[:],
            in_=core_all[:, lo * A : hi * A].rearrange("p (t a) -> p a t", a=A),
            axis=mybir.AxisListType.X,
        )
        dst2 = ms_c2 if i == 0 else msp_c2
        nc.vector.reduce_sum(
            out=dst2[:],
            in_=sq_all[:, lo * A : hi * A].rearrange("p (t a) -> p a t", a=A),
            axis=mybir.AxisListType.X,
        )
    nc.vector.tensor_tensor(out=ms_c[:], in0=ms_c[:], in1=msp_c[:], op=mybir.AluOpType.add)
    nc.vector.tensor_tensor(out=ms_c2[:], in0=ms_c2[:], in1=msp_c2[:], op=mybir.AluOpType.add)
    ones128 = const.tile([P, 1], f32)
    nc.vector.memset(ones128[:], 1.0)
    statc = psum2.tile([1, A], f32, name="statc")
    statc2 = psum2.tile([1, A], f32, name="statc2")
    nc.tensor.matmul(out=statc[:], lhsT=ones128[:], rhs=ms_c[:], start=True, stop=True)
    nc.tensor.matmul(out=statc2[:], lhsT=ones128[:], rhs=ms_c2[:], start=True, stop=True)

    # ---- AllReduce BN2 stats ----
    s2_pack = persist.tile([1, 2 * A], f32)
    nc.vector.tensor_copy(out=s2_pack[:, 0:A], in_=statc[:])
    nc.vector.tensor_copy(out=s2_pack[:, A : 2 * A], in_=statc2[:])
    ar2_in = dram.tile([1, 2 * A], f32)
    ar2_out = dram.tile([1, 2 * A], f32, addr_space="Shared")
    nc.sync.dma_start(out=ar2_in[:], in_=s2_pack[:])
    nc.gpsimd.collective_compute(
        "AllReduce", mybir.AluOpType.add, replica_groups=rg,
        ins=[ar2_in[:].opt()], outs=[ar2_out[:].opt()],
    )
    s2g = persist.tile([1, 2 * A], f32)
    nc.sync.dma_start(out=s2g[:], in_=ar2_out[:])
    sum2g_ap = s2g[:, 0:A]
    sq2g_ap = s2g[:, A : 2 * A]

    mean2 = coef.tile([1, A], f32)
    nc.scalar.mul(mean2[:], sum2g_ap, inv_n)
    ex22 = coef.tile([1, A], f32)
    nc.scalar.mul(ex22[:], sq2g_ap, inv_n)
    msq2 = coef.tile([1, A], f32)
    nc.vector.tensor_tensor(out=msq2[:], in0=mean2[:], in1=mean2[:], op=mybir.AluOpType.mult)
    var2 = coef.tile([1, A], f32)
    nc.vector.tensor_tensor(out=var2[:], in0=ex22[:], in1=msq2[:], op=mybir.AluOpType.subtract)
    nc.vector.tensor_scalar_add(var2[:], var2[:], EPS)
    lnv2 = coef.tile([1, A], f32)
    nc.scalar.activation(lnv2[:], var2[:], mybir.ActivationFunctionType.Ln)
    rstd2 = coef.tile([1, A], f32)
    nc.scalar.activation(rstd2[:], lnv2[:], mybir.ActivationFunctionType.Exp, scale=-0.5)
    a2c2 = coef.tile([1, 2 * A], f32)
    nc.vector.tensor_tensor(out=a2c2[:, 0:A], in0=gamma2_sb[:], in1=rstd2[:], op=mybir.AluOpType.mult)
    ma2 = coef.tile([1, A], f32)
    nc.vector.tensor_tensor(out=ma2[:], in0=mean2[:], in1=a2c2[:, 0:A], op=mybir.AluOpType.mult)
    nc.vector.tensor_tensor(out=a2c2[:, A : 2 * A], in0=beta2_sb[:], in1=ma2[:], op=mybir.AluOpType.subtract)

    bc2p = psum_b.tile([P, 2 * A], f32)
    nc.tensor.matmul(out=bc2p[:], lhsT=ones1[:], rhs=a2c2[:], start=True, stop=True)
    A2C2 = persist.tile([P, 2 * A], f32)
    nc.vector.tensor_copy(out=A2C2[:], in_=bc2p[:])

    # ---------------- phase 3: BN2 apply + softplus + store ----------------
    a2_full = A2C2[:, 0:A]
    c2_full = A2C2[:, A : 2 * A]
    for lo, hi in chunks:
        nt = hi - lo
        corev = core_all[:, lo * A : hi * A].rearrange("p (t a) -> p t a", a=A)
        a2_per = a2_full.unsqueeze(1).to_broadcast([P, nt, A])
        c2_per = c2_full.unsqueeze(1).to_broadcast([P, nt, A])
        nc.vector.tensor_tensor(out=corev, in0=corev, in1=a2_per, op=mybir.AluOpType.mult)
        nc.vector.tensor_tensor(out=corev, in0=corev, in1=c2_per, op=mybir.AluOpType.add)
    for lo, hi in chunks:
        nc.scalar.activation(
            core_all[:, lo * A : hi * A],
            core_all[:, lo * A : hi * A],
            mybir.ActivationFunctionType.Exp,
        )
    for lo, hi in chunks:
        nc.scalar.activation(
            core_all[:, lo * A : hi * A],
            core_all[:, lo * A : hi * A],
            mybir.ActivationFunctionType.Ln,
            bias=1.0,
        )
        nc.sync.dma_start(
            out=out_d[lo * P :, :].rearrange("(t p) a -> p t a", p=P)[:, 0 : hi - lo, :],
            in_=core_all[:, lo * A : hi * A].rearrange("p (t a) -> p t a", a=A),
        )

    es2.close()
    es_main.close()
    tc.__exit__(None, None, None)
    nc_.compile()
    return nc_


def make_in_maps(inputs, n=N, m=M, cores=CORES):
    """Host-side sharding/layout prep (index reshuffling + dtype conversion only)."""
    ns = n // cores
    t_tiles = (ns + P - 1) // P
    npad = t_tiles * P

    atom = np.asarray(inputs["atom_in_fea"], np.float32)
    nbr = np.asarray(inputs["nbr_fea"], np.float32).reshape(n, m * B)
    idx = np.asarray(inputs["nbr_fea_idx"])
    bwi = np.asarray(inputs["bond_weights_i"], np.float32)
    bwj = np.asarray(inputs["bond_weights_j"], np.float32)
    W = np.asarray(inputs["W"], np.float32)
    g1 = np.asarray(inputs["gamma1"], np.float32)
    b1 = np.asarray(inputs["beta1"], np.float32)
    g2 = np.asarray(inputs["gamma2"], np.float32)
    b2 = np.asarray(inputs["beta2"], np.float32)

    Wt = np.ascontiguousarray(W.T)  # [F, J]
    wt1 = np.ascontiguousarray(Wt[0:J, :])
    wt2 = np.ascontiguousarray(Wt[J:F, :])
    g1b1 = np.stack([g1, b1]).astype(np.float32)
    g2b2 = np.stack([g2, b2]).astype(np.float32)

    valid = np.zeros((npad,), np.float32)
    valid[:ns] = 1.0
    validT = np.ascontiguousarray(valid.reshape(t_tiles, P).T)

    in_maps = []
    for c in range(cores):
        lo, hi = c * ns, (c + 1) * ns
        pad = npad - ns

        def padrows(x):
            return np.concatenate([x, np.zeros((pad,) + x.shape[1:], x.dtype)], 0) if pad else x

        idx_c = np.concatenate([idx[lo:hi], np.zeros((pad, m), idx.dtype)], 0) if pad else idx[lo:hi]

        in_maps.append(
            {
                "atab": atom.astype(ml_dtypes.bfloat16),
                "aself": padrows(atom[lo:hi]),
                "nbr": padrows(nbr[lo:hi]),
                "bwi": padrows(bwi[lo:hi]),
                "bwj": padrows(bwj[lo:hi]),
                "idx": np.ascontiguousarray(idx_c.astype(np.int32)),
                "validT": validT,
                "wt1": wt1,
                "wt2": wt2,
                "g1b1": g1b1,
                "g2b2": g2b2,
                "ident": np.eye(P, dtype=np.float32),
            }
        )
    return in_maps


_GRAPH_CACHE = {}


def _get_graph():
    if "nc" not in _GRAPH_CACHE:
        _GRAPH_CACHE["nc"] = build_graph()
    return _GRAPH_CACHE["nc"]


def run(inputs, trace=False, **kw):
    nc = _get_graph()
    in_maps = make_in_maps(inputs)
    res = run_bass_kernel_spmd(nc, in_maps, core_ids=list(range(CORES)), trace=trace, **kw)
    ns = N // CORES
    out = np.concatenate([res.results[c]["out"][:ns] for c in range(CORES)], 0)
    return out.astype(np.float32), res


def kernel(**inputs) -> np.ndarray:
    out, _ = run(inputs, trace=False)
    return out
